# revision 6
# baseline (speedup 1.0000x reference)
"""Trainium2 Bass kernel: per-element maximization of the lognormal-CDF
surplus  s(d) = bid*(1-d)*Phi((ln(d*bid)-mu)/sigma),  d in [0,1].

Algorithm: the reference runs 20 golden-section iterations on s(d) (two
surplus evaluations per iteration).  s is log-concave in d (product of
log-concave factors composed with concave increasing maps), so s' crosses
zero exactly once and the argmax can instead be found by BISECTION ON THE
SIGN OF s'(d) - one evaluation per iteration, 0.5x interval shrink per
iteration (vs 0.618x for GSS):

  s'(d) >= 0  <=>  (1-d) * B * phi0 * e^{-z^2} >= d * (1 + erf z)
     z = (ln d - m) * B,  m = mu - ln bid,  B = 1/(sigma*sqrt2),
     phi0 = 2/sqrt(pi)

Validated against the reference output: rel-L2 plateaus at ~5.2e-3 for
K >= 11 (the residual is reference fp32 (1+erf) quantization noise on
deep-tail elements, not bisection resolution) - comfortably under the
2e-2 gate, and robust to 1e-3 activation-table error.

Implementation notes (per [128, FD] chunk):
  - Only the interval MIDPOINT D is tracked:  D' = D + (delta>=0)*w - w/2,
    one fused custom-DVE op (GSS_STEPD); the final STEPD emits the answer.
  - z' = (ln D - m)/sigma; erf(z) via ACT Erf with input scale 1/sqrt2;
    the gaussian side folds B*phi0 into the ACT Exp bias:
      P' = Exp(-(0.5 z'^2 + ln sigma) + ln(phi0/sqrt2)) = B*phi0*e^{-z^2}
    so the per-iter ACT tables are {Ln, Exp} (one set) + {Erf}: 2 swaps.
  - Custom fused DVE ops (registered at import): sq-scale-add for the Exp
    argument, the step update, and iteration-0 specializations (D_0 = 0.5
    is a compile-time constant, so iter 0 needs no Ln and folds M=0.5
    into the Exp bias).
  - Work is split DVE / Pool (gpsimd) / ACT; the delta & p_r ops alternate
    between DVE and Pool by chunk parity to balance the two queues.
"""
import sys

sys.path.insert(0, "/opt/trn_rl_repo")

import numpy as np

N_TOTAL = 16777216
N_CORES = 8
N_PER_CORE = N_TOTAL // N_CORES  # 2097152
P = 128
FD = 1024
GROUP = 4
K_ITERS = 10

LN_HALF = float(np.log(0.5))
INV_SQRT2 = float(1.0 / np.sqrt(2.0))
# ln(phi0/sqrt2), phi0 = 2/sqrt(pi)
LNPHI = float(np.log(2.0 / np.sqrt(np.pi)) - 0.5 * np.log(2.0))
LNPHI_HALF = float(LNPHI + np.log(0.5))  # iter-0: M = 0.5 folded in

_ops_registered = {}


def _register_ops():
    """Register the fused custom-DVE ops (documented extension point:
    dve_ops.OPS + _SUB_OPCODE_FOR_NAME + CUSTOM_DVE_SPECS). uops_sha is
    computed here the same way DveOp.compile derives it."""
    if _ops_registered:
        return _ops_registered
    import concourse.dve_ops as dve_ops
    from concourse.dve_ops import DveOp, OPS
    from concourse.dve_spec import Spec, Src0, Src1, C0, C2, Zero, sq, lower
    from concourse.dve_spec import _has_src1 as has_src1
    from concourse.dve_uop import DveOpSpec

    def ref_sqsa(in0, in1, s0, s1, imm2):
        return (np.float32(s0) * in0.astype(np.float32) ** 2 + in1).astype(np.float32)

    def ref_stepd(in0, in1, s0, s1, imm2):
        return (in1 + (in0 >= 0).astype(np.float32) * np.float32(imm2)
                + np.float32(s0)).astype(np.float32)

    def ref_z0(in0, in1, s0, s1, imm2):
        return ((np.float32(s0) - in0.astype(np.float32)) * in1).astype(np.float32)

    def ref_step0(in0, in1, s0, s1, imm2):
        return ((in0 >= 0).astype(np.float32) * np.float32(imm2)
                + np.float32(s0)).astype(np.float32)

    defs = [
        # v2 = 0.5*z'^2 + ln(sigma)
        ("GSS_SQSA", sq(Src0) * C0 + Src1, ref_sqsa),
        # D' = D + (delta >= 0)*w - w/2
        ("GSS_STEPD", Src1 + (Src0 >= Zero) * C2 + C0, ref_stepd),
        # z0 = (ln(0.5) - m) * (1/sigma)
        ("GSS_Z0", (C0 - Src0) * Src1, ref_z0),
        # D1 = (delta >= 0)*0.5 + 0.25
        ("GSS_STEP0", (Src0 >= Zero) * C2 + C0, ref_step0),
    ]
    for name, body, ref in defs:
        if name in dve_ops._SUB_OPCODE_FOR_NAME:
            _ops_registered[name] = next(o for o in OPS if o.name == name)
            continue
        row = dve_ops._CUSTOM_DVE_ROW_BASE + len(OPS)
        assert row < 0x20
        spec = Spec(body=body, reference=ref)
        shas = {}
        for ver in ("v3", "v4"):
            uops = lower(spec, ver=ver)
            shas[ver] = DveOpSpec(
                name=name, opcode=row, uops=uops, rd1_en=has_src1(spec)
            ).sha(ver)
        op = DveOp(name, spec, subdim=False, uops_sha=shas)
        OPS.append(op)
        dve_ops._SUB_OPCODE_FOR_NAME[name] = row
        dve_ops.CUSTOM_DVE_SPECS[name] = spec
        _ops_registered[name] = op
    return _ops_registered


def _build_nc(n_per_core, fd, group_size):
    import concourse.bass as bass  # noqa: F401
    import concourse.bacc as bacc
    import concourse.mybir as mybir
    import concourse.tile as tile

    ops = _register_ops()
    SQSA, STEPD, Z0, STEP0 = (
        ops["GSS_SQSA"], ops["GSS_STEPD"], ops["GSS_Z0"], ops["GSS_STEP0"]
    )

    AF = mybir.ActivationFunctionType
    ALU = mybir.AluOpType
    dt = mybir.dt.float32

    n_chunks = n_per_core // (P * fd)
    assert n_chunks * P * fd == n_per_core

    nc = bacc.Bacc(None, target_bir_lowering=False)

    def register_const(value: float):
        if (dt, value) in nc.const_aps.aps:
            return
        t = nc.alloc_sbuf_tensor(f"const-f32-c{len(nc.const_aps.aps)}", [128, 1], dt)
        nc.gpsimd.memset(t.ap(), value)
        nc.const_aps.aps[(dt, value)] = t.ap()

    for v in (0.0, LNPHI, LNPHI_HALF):
        register_const(float(v))
    nc.all_engine_barrier()

    params = nc.declare_dram_parameter("params", [n_per_core, 2], dt, isOutput=False)
    bids = nc.declare_dram_parameter("bids", [n_per_core], dt, isOutput=False)
    out = nc.declare_dram_parameter("out", [n_per_core], dt, isOutput=True)

    params_v = params.rearrange("(g p f) c -> g p (f c)", p=P, f=fd)
    bids_v = bids.rearrange("(g p f) -> g p f", p=P, f=fd)
    out_v = out.rearrange("(g p f) -> g p f", p=P, f=fd)

    with tile.TileContext(nc) as tc:
        with (
            tc.tile_pool(name="st_d", bufs=group_size + 1) as p_d,
            tc.tile_pool(name="st_m", bufs=group_size + 1) as p_m,
            tc.tile_pool(name="st_rs", bufs=group_size + 1) as p_rs,
            tc.tile_pool(name="st_ls", bufs=group_size + 1) as p_ls,
            tc.tile_pool(name="s1", bufs=group_size + 1) as p_s1,
            tc.tile_pool(name="s2", bufs=group_size + 1) as p_s2,
            tc.tile_pool(name="s3", bufs=group_size + 1) as p_s3,
            tc.tile_pool(name="s4", bufs=group_size + 1) as p_s4,
            tc.tile_pool(name="pload", bufs=2) as p_pl,
        ):
            for g0 in range(0, n_chunks, group_size):
                members = []
                # ---- per-chunk setup ----
                for gi in range(g0, min(g0 + group_size, n_chunks)):
                    D = p_d.tile([P, fd], dt, tag="D")
                    m = p_m.tile([P, fd], dt, tag="m")
                    rs = p_rs.tile([P, fd], dt, tag="rs")
                    ls = p_ls.tile([P, fd], dt, tag="ls")
                    # bid -> m (staging), then ln in place
                    nc.sync.dma_start(m[:], bids_v[gi])
                    nc.scalar.activation(m[:], m[:], AF.Ln)
                    for h in range(2):
                        pl = p_pl.tile([P, fd], dt, tag="pl")
                        nc.sync.dma_start(pl[:], params_v[gi, :, h * fd:(h + 1) * fd])
                        plv = pl.rearrange("p (f c) -> p f c", c=2)
                        half = slice(h * (fd // 2), (h + 1) * (fd // 2))
                        # ls = ln(sigma); rs = 1/sigma; m = mu - ln(bid)
                        nc.scalar.activation(ls[:, half], plv[:, :, 1], AF.Ln)
                        nc.vector.reciprocal_approx_fast(
                            out=rs[:, half], in_=plv[:, :, 1]
                        )
                        nc.vector.tensor_sub(m[:, half], plv[:, :, 0], m[:, half])
                    members.append((gi, D, m, rs, ls))

                # ---- iteration 0: D_0 = 0.5 (compile-time constant) ----
                scratch = {}
                for gi, D, m, rs, ls in members:
                    s1 = p_s1.tile([P, fd], dt, tag="s1")
                    s2 = p_s2.tile([P, fd], dt, tag="s2")
                    s3 = p_s3.tile([P, fd], dt, tag="s3")
                    s4 = p_s4.tile([P, fd], dt, tag="s4")
                    scratch[gi] = (s1, s2, s3, s4)
                    nc.vector._custom_dve(Z0, out=s1[:], in0=m[:], in1=rs[:],
                                          s0=LN_HALF)
                for gi, D, m, rs, ls in members:
                    s1, s2, s3, s4 = scratch[gi]
                    nc.scalar.activation(s4[:], s1[:], AF.Erf, scale=INV_SQRT2)
                for gi, D, m, rs, ls in members:
                    s1, s2, s3, s4 = scratch[gi]
                    nc.vector._custom_dve(SQSA, out=s2[:], in0=s1[:], in1=ls[:],
                                          s0=0.5)
                for gi, D, m, rs, ls in members:
                    s1, s2, s3, s4 = scratch[gi]
                    # p_l = P' * 0.5 (M folded into bias)
                    nc.scalar.activation(s3[:], s2[:], AF.Exp, scale=-1.0,
                                         bias=LNPHI_HALF)
                for gi, D, m, rs, ls in members:
                    s1, s2, s3, s4 = scratch[gi]
                    # p_r = (E+1)*0.5
                    nc.vector.tensor_scalar(s4[:], s4[:], 1.0, 0.5,
                                            op0=ALU.add, op1=ALU.mult)
                for gi, D, m, rs, ls in members:
                    s1, s2, s3, s4 = scratch[gi]
                    nc.gpsimd.tensor_sub(s4[:], s3[:], s4[:])
                for gi, D, m, rs, ls in members:
                    s1, s2, s3, s4 = scratch[gi]
                    nc.vector._custom_dve(STEP0, out=D[:], in0=s4[:],
                                          s0=0.25, imm2=0.5)

                # ---- iterations 1..K-1 ----
                for k in range(1, K_ITERS):
                    w = float(2.0 ** -(k + 1))
                    s0 = float(-(2.0 ** -(k + 2)))
                    for gi, D, m, rs, ls in members:
                        s1 = p_s1.tile([P, fd], dt, tag="s1")
                        s2 = p_s2.tile([P, fd], dt, tag="s2")
                        s3 = p_s3.tile([P, fd], dt, tag="s3")
                        s4 = p_s4.tile([P, fd], dt, tag="s4")
                        scratch[gi] = (s1, s2, s3, s4)
                        nc.scalar.activation(s1[:], D[:], AF.Ln)
                    for gi, D, m, rs, ls in members:
                        s1, s2, s3, s4 = scratch[gi]
                        # M = 1 - D  (Copy is in every ACT table set)
                        nc.scalar.activation(s3[:], D[:], AF.Copy, scale=-1.0,
                                             bias=1.0)
                    for gi, D, m, rs, ls in members:
                        s1, s2, s3, s4 = scratch[gi]
                        nc.gpsimd.tensor_sub(s1[:], s1[:], m[:])
                    for gi, D, m, rs, ls in members:
                        s1, s2, s3, s4 = scratch[gi]
                        nc.vector.tensor_mul(s1[:], s1[:], rs[:])
                    for gi, D, m, rs, ls in members:
                        s1, s2, s3, s4 = scratch[gi]
                        nc.scalar.activation(s4[:], s1[:], AF.Erf, scale=INV_SQRT2)
                    for gi, D, m, rs, ls in members:
                        s1, s2, s3, s4 = scratch[gi]
                        nc.vector._custom_dve(SQSA, out=s2[:], in0=s1[:],
                                              in1=ls[:], s0=0.5)
                    for gi, D, m, rs, ls in members:
                        s1, s2, s3, s4 = scratch[gi]
                        nc.scalar.activation(s2[:], s2[:], AF.Exp, scale=-1.0,
                                             bias=LNPHI)
                    for gi, D, m, rs, ls in members:
                        s1, s2, s3, s4 = scratch[gi]
                        # p_l = P' * M
                        nc.gpsimd.tensor_mul(s3[:], s2[:], s3[:])
                    for gi, D, m, rs, ls in members:
                        s1, s2, s3, s4 = scratch[gi]
                        # p_r = (E+1)*D
                        nc.vector.scalar_tensor_tensor(s4[:], s4[:], 1.0, D[:],
                                                       op0=ALU.add, op1=ALU.mult)
                    for i, (gi, D, m, rs, ls) in enumerate(members):
                        s1, s2, s3, s4 = scratch[gi]
                        # delta = p_l - p_r (alternates Pool/DVE to balance)
                        eng = nc.gpsimd if (i + k) % 2 == 0 else nc.vector
                        eng.tensor_sub(s4[:], s3[:], s4[:])
                    for gi, D, m, rs, ls in members:
                        s1, s2, s3, s4 = scratch[gi]
                        nc.vector._custom_dve(STEPD, out=D[:], in0=s4[:],
                                              in1=D[:], s0=s0, imm2=w)

                # ---- store (the last STEPD already wrote the midpoint) ----
                for gi, D, m, rs, ls in members:
                    nc.sync.dma_start(out_v[gi], D[:])

    nc.finalize()
    return nc


_CACHED = {}


def _get_nc(n_per_core, fd=FD, group_size=GROUP):
    key = (n_per_core, fd, group_size)
    if key not in _CACHED:
        _CACHED[key] = _build_nc(n_per_core, fd, group_size)
    return _CACHED[key]


def kernel(params: np.ndarray, bid_prices: np.ndarray) -> np.ndarray:
    from concourse.bass_utils import run_bass_kernel_spmd

    params = np.ascontiguousarray(params, dtype=np.float32)
    bid_prices = np.ascontiguousarray(bid_prices, dtype=np.float32)
    n = bid_prices.shape[0]
    n_per_core = n // N_CORES

    nc = _get_nc(n_per_core)

    in_maps = []
    for i in range(N_CORES):
        sl = slice(i * n_per_core, (i + 1) * n_per_core)
        in_maps.append({"params": params[sl], "bids": bid_prices[sl]})

    res = run_bass_kernel_spmd(nc, in_maps, core_ids=list(range(N_CORES)))
    return np.concatenate([r["out"] for r in res.results], axis=0)


if __name__ == "__main__":
    rng = np.random.RandomState(0)
    n = N_TOTAL
    params = np.stack(
        [rng.randn(n).astype(np.float32),
         rng.uniform(0.2, 1.5, n).astype(np.float32)], axis=-1
    )
    bids = rng.uniform(0.1, 10.0, n).astype(np.float32)
    out = kernel(params=params, bid_prices=bids)
    print("out", out.shape, out.dtype, out[:8])


# revision 8
# speedup vs baseline: 2.1092x; 2.1092x over previous
"""Trainium2 Bass kernel: per-element maximization of the lognormal-CDF
surplus  s(d) = bid*(1-d)*Phi((ln(d*bid)-mu)/sigma),  d in [0,1].

Algorithm: the reference runs 20 golden-section iterations on s(d) (two
surplus evaluations per iteration).  s is log-concave in d (product of
log-concave factors composed with concave increasing maps), so s' crosses
zero exactly once and the argmax can instead be found by BISECTION ON THE
SIGN OF s'(d) - one evaluation per iteration, 0.5x interval shrink per
iteration (vs 0.618x for GSS):

  s'(d) >= 0  <=>  (1-d) * B * phi0 * e^{-z^2} >= d * (1 + erf z)
     z = (ln d - m) * B,  m = mu - ln bid,  B = 1/(sigma*sqrt2),
     phi0 = 2/sqrt(pi)

Validated against the reference output: rel-L2 plateaus at ~5.2e-3 for
K >= 11 (the residual is reference fp32 (1+erf) quantization noise on
deep-tail elements, not bisection resolution) - comfortably under the
2e-2 gate, and robust to 1e-3 activation-table error.

Implementation notes (per [128, FD] chunk):
  - Only the interval MIDPOINT D is tracked:  D' = D + (delta>=0)*w - w/2,
    one fused custom-DVE op (GSS_STEPD); the final STEPD emits the answer.
  - z' = (ln D - m)/sigma; erf(z) via ACT Erf with input scale 1/sqrt2;
    the gaussian side folds B*phi0 into the ACT Exp bias:
      P' = Exp(-(0.5 z'^2 + ln sigma) + ln(phi0/sqrt2)) = B*phi0*e^{-z^2}
    so the per-iter ACT tables are {Ln, Exp} (one set) + {Erf}: 2 swaps.
  - Custom fused DVE ops (registered at import): sq-scale-add for the Exp
    argument, the step update, and iteration-0 specializations (D_0 = 0.5
    is a compile-time constant, so iter 0 needs no Ln and folds M=0.5
    into the Exp bias).
  - Work is split DVE / Pool (gpsimd) / ACT; the delta & p_r ops alternate
    between DVE and Pool by chunk parity to balance the two queues.
"""
import sys

sys.path.insert(0, "/opt/trn_rl_repo")

import numpy as np

N_TOTAL = 16777216
N_CORES = 8
N_PER_CORE = N_TOTAL // N_CORES  # 2097152
P = 128
FD = 1024
GROUP = 4
K_ITERS = 8

LN_HALF = float(np.log(0.5))
INV_SQRT2 = float(1.0 / np.sqrt(2.0))
# ln(phi0/sqrt2), phi0 = 2/sqrt(pi)
LNPHI = float(np.log(2.0 / np.sqrt(np.pi)) - 0.5 * np.log(2.0))
LNPHI_HALF = float(LNPHI + np.log(0.5))  # iter-0: M = 0.5 folded in

_ops_registered = {}


def _register_ops():
    """Register the fused custom-DVE ops (documented extension point:
    dve_ops.OPS + _SUB_OPCODE_FOR_NAME + CUSTOM_DVE_SPECS). uops_sha is
    computed here the same way DveOp.compile derives it."""
    if _ops_registered:
        return _ops_registered
    import concourse.dve_ops as dve_ops
    from concourse.dve_ops import DveOp, OPS
    from concourse.dve_spec import Spec, Src0, Src1, C0, C2, Zero, sq, lower
    from concourse.dve_spec import _has_src1 as has_src1
    from concourse.dve_uop import DveOpSpec

    def ref_sqsa(in0, in1, s0, s1, imm2):
        return (np.float32(s0) * in0.astype(np.float32) ** 2 + in1).astype(np.float32)

    def ref_stepd(in0, in1, s0, s1, imm2):
        return (in1 + (in0 >= 0).astype(np.float32) * np.float32(imm2)
                + np.float32(s0)).astype(np.float32)

    def ref_z0(in0, in1, s0, s1, imm2):
        return ((np.float32(s0) - in0.astype(np.float32)) * in1).astype(np.float32)

    def ref_step0(in0, in1, s0, s1, imm2):
        return ((in0 >= 0).astype(np.float32) * np.float32(imm2)
                + np.float32(s0)).astype(np.float32)

    defs = [
        # v2 = 0.5*z'^2 + ln(sigma)
        ("GSS_SQSA", sq(Src0) * C0 + Src1, ref_sqsa),
        # D' = D + (delta >= 0)*w - w/2
        ("GSS_STEPD", Src1 + (Src0 >= Zero) * C2 + C0, ref_stepd),
        # z0 = (ln(0.5) - m) * (1/sigma)
        ("GSS_Z0", (C0 - Src0) * Src1, ref_z0),
        # D1 = (delta >= 0)*0.5 + 0.25
        ("GSS_STEP0", (Src0 >= Zero) * C2 + C0, ref_step0),
    ]
    for name, body, ref in defs:
        if name in dve_ops._SUB_OPCODE_FOR_NAME:
            _ops_registered[name] = next(o for o in OPS if o.name == name)
            continue
        row = dve_ops._CUSTOM_DVE_ROW_BASE + len(OPS)
        assert row < 0x20
        spec = Spec(body=body, reference=ref)
        shas = {}
        for ver in ("v3", "v4"):
            uops = lower(spec, ver=ver)
            shas[ver] = DveOpSpec(
                name=name, opcode=row, uops=uops, rd1_en=has_src1(spec)
            ).sha(ver)
        op = DveOp(name, spec, subdim=False, uops_sha=shas)
        OPS.append(op)
        dve_ops._SUB_OPCODE_FOR_NAME[name] = row
        dve_ops.CUSTOM_DVE_SPECS[name] = spec
        _ops_registered[name] = op
    return _ops_registered


def _build_nc(n_per_core, fd, group_size):
    import concourse.bass as bass  # noqa: F401
    import concourse.bacc as bacc
    import concourse.mybir as mybir
    import concourse.tile as tile

    ops = _register_ops()
    SQSA, STEPD, Z0, STEP0 = (
        ops["GSS_SQSA"], ops["GSS_STEPD"], ops["GSS_Z0"], ops["GSS_STEP0"]
    )

    AF = mybir.ActivationFunctionType
    ALU = mybir.AluOpType
    dt = mybir.dt.float32
    dtb = mybir.dt.bfloat16

    n_chunks = n_per_core // (P * fd)
    assert n_chunks * P * fd == n_per_core

    nc = bacc.Bacc(None, target_bir_lowering=False)

    def register_const(value: float):
        if (dt, value) in nc.const_aps.aps:
            return
        t = nc.alloc_sbuf_tensor(f"const-f32-c{len(nc.const_aps.aps)}", [128, 1], dt)
        nc.gpsimd.memset(t.ap(), value)
        nc.const_aps.aps[(dt, value)] = t.ap()

    for v in (0.0, LNPHI, LNPHI_HALF):
        register_const(float(v))
    nc.all_engine_barrier()

    params = nc.declare_dram_parameter("params", [n_per_core, 2], dt, isOutput=False)
    bids = nc.declare_dram_parameter("bids", [n_per_core], dt, isOutput=False)
    out = nc.declare_dram_parameter("out", [n_per_core], dt, isOutput=True)

    params_v = params.rearrange("(g p f) c -> g p (f c)", p=P, f=fd)
    bids_v = bids.rearrange("(g p f) -> g p f", p=P, f=fd)
    out_v = out.rearrange("(g p f) -> g p f", p=P, f=fd)

    with tile.TileContext(nc) as tc:
        with (
            tc.tile_pool(name="st_d", bufs=group_size + 1) as p_d,
            tc.tile_pool(name="st_m", bufs=group_size + 1) as p_m,
            tc.tile_pool(name="st_rs", bufs=group_size + 1) as p_rs,
            tc.tile_pool(name="st_ls", bufs=group_size + 1) as p_ls,
            tc.tile_pool(name="s1", bufs=group_size + 1) as p_s1,
            tc.tile_pool(name="s2", bufs=group_size + 1) as p_s2,
            tc.tile_pool(name="s3", bufs=group_size + 1) as p_s3,
            tc.tile_pool(name="s4", bufs=group_size + 1) as p_s4,
            tc.tile_pool(name="s2b", bufs=group_size + 1) as p_s2b,
            tc.tile_pool(name="s4b", bufs=group_size + 1) as p_s4b,
            tc.tile_pool(name="pload", bufs=2) as p_pl,
        ):
            for g0 in range(0, n_chunks, group_size):
                members = []
                # ---- per-chunk setup ----
                for gi in range(g0, min(g0 + group_size, n_chunks)):
                    D = p_d.tile([P, fd], dt, tag="D")
                    m = p_m.tile([P, fd], dt, tag="m")
                    rs = p_rs.tile([P, fd], dt, tag="rs")
                    ls = p_ls.tile([P, fd], dt, tag="ls")
                    # bid -> m (staging), then ln in place
                    nc.sync.dma_start(m[:], bids_v[gi])
                    nc.scalar.activation(m[:], m[:], AF.Ln)
                    for h in range(2):
                        pl = p_pl.tile([P, fd], dt, tag="pl")
                        nc.sync.dma_start(pl[:], params_v[gi, :, h * fd:(h + 1) * fd])
                        plv = pl.rearrange("p (f c) -> p f c", c=2)
                        half = slice(h * (fd // 2), (h + 1) * (fd // 2))
                        # ls = ln(sigma); rs = 1/sigma; m = mu - ln(bid)
                        nc.scalar.activation(ls[:, half], plv[:, :, 1], AF.Ln)
                        nc.vector.reciprocal_approx_fast(
                            out=rs[:, half], in_=plv[:, :, 1]
                        )
                        nc.vector.tensor_sub(m[:, half], plv[:, :, 0], m[:, half])
                    members.append((gi, D, m, rs, ls))

                # ---- iteration 0: D_0 = 0.5 (compile-time constant) ----
                scratch = {}
                for gi, D, m, rs, ls in members:
                    s1 = p_s1.tile([P, fd], dt, tag="s1")
                    s2 = p_s2.tile([P, fd], dt, tag="s2")
                    s3 = p_s3.tile([P, fd], dt, tag="s3")
                    s4 = p_s4.tile([P, fd], dt, tag="s4")
                    scratch[gi] = (s1, s2, s3, s4)
                    nc.vector._custom_dve(Z0, out=s1[:], in0=m[:], in1=rs[:],
                                          s0=LN_HALF)
                for gi, D, m, rs, ls in members:
                    s1, s2, s3, s4 = scratch[gi]
                    nc.scalar.activation(s4[:], s1[:], AF.Erf, scale=INV_SQRT2)
                for gi, D, m, rs, ls in members:
                    s1, s2, s3, s4 = scratch[gi]
                    nc.vector._custom_dve(SQSA, out=s2[:], in0=s1[:], in1=ls[:],
                                          s0=0.5)
                for gi, D, m, rs, ls in members:
                    s1, s2, s3, s4 = scratch[gi]
                    # p_l = P' * 0.5 (M folded into bias)
                    nc.scalar.activation(s3[:], s2[:], AF.Exp, scale=-1.0,
                                         bias=LNPHI_HALF)
                for gi, D, m, rs, ls in members:
                    s1, s2, s3, s4 = scratch[gi]
                    # p_r = (E+1)*0.5
                    nc.vector.tensor_scalar(s4[:], s4[:], 1.0, 0.5,
                                            op0=ALU.add, op1=ALU.mult)
                for gi, D, m, rs, ls in members:
                    s1, s2, s3, s4 = scratch[gi]
                    nc.vector.tensor_sub(s4[:], s3[:], s4[:])
                for gi, D, m, rs, ls in members:
                    s1, s2, s3, s4 = scratch[gi]
                    nc.vector._custom_dve(STEP0, out=D[:], in0=s4[:],
                                          s0=0.25, imm2=0.5)

                # ---- iterations 1..K-1 ----
                for k in range(1, K_ITERS):
                    w = float(2.0 ** -(k + 1))
                    s0 = float(-(2.0 ** -(k + 2)))
                    for gi, D, m, rs, ls in members:
                        s1 = p_s1.tile([P, fd], dt, tag="s1")
                        s2 = p_s2.tile([P, fd], dt, tag="s2")
                        s2b = p_s2b.tile([P, fd], dtb, tag="s2b")
                        s3 = p_s3.tile([P, fd], dtb, tag="s3")
                        s4 = p_s4.tile([P, fd], dt, tag="s4")
                        s4b = p_s4b.tile([P, fd], dtb, tag="s4b")
                        scratch[gi] = (s1, s2, s2b, s3, s4, s4b)
                        nc.scalar.activation(s1[:], D[:], AF.Ln)
                    for gi, D, m, rs, ls in members:
                        s1, s2, s2b, s3, s4, s4b = scratch[gi]
                        # M = 1 - D  (Copy is in every ACT table set), bf16
                        nc.scalar.activation(s3[:], D[:], AF.Copy, scale=-1.0,
                                             bias=1.0)
                    for gi, D, m, rs, ls in members:
                        s1, s2, s2b, s3, s4, s4b = scratch[gi]
                        nc.vector.tensor_sub(s1[:], s1[:], m[:])
                    for gi, D, m, rs, ls in members:
                        s1, s2, s2b, s3, s4, s4b = scratch[gi]
                        nc.vector.tensor_mul(s1[:], s1[:], rs[:])
                    for gi, D, m, rs, ls in members:
                        s1, s2, s2b, s3, s4, s4b = scratch[gi]
                        nc.scalar.activation(s4[:], s1[:], AF.Erf, scale=INV_SQRT2)
                    for gi, D, m, rs, ls in members:
                        s1, s2, s2b, s3, s4, s4b = scratch[gi]
                        nc.vector._custom_dve(SQSA, out=s2[:], in0=s1[:],
                                              in1=ls[:], s0=0.5)
                    for gi, D, m, rs, ls in members:
                        s1, s2, s2b, s3, s4, s4b = scratch[gi]
                        # P' in bf16
                        nc.scalar.activation(s2b[:], s2[:], AF.Exp, scale=-1.0,
                                             bias=LNPHI)
                    for gi, D, m, rs, ls in members:
                        s1, s2, s2b, s3, s4, s4b = scratch[gi]
                        # p_l = P' * M  (bf16 x bf16 -> bf16: 2x DVE mode)
                        nc.vector.tensor_mul(s3[:], s2b[:], s3[:])
                    for gi, D, m, rs, ls in members:
                        s1, s2, s2b, s3, s4, s4b = scratch[gi]
                        # p_r = (E+1)*D -> bf16
                        nc.vector.scalar_tensor_tensor(s4b[:], s4[:], 1.0, D[:],
                                                       op0=ALU.add, op1=ALU.mult)
                    for gi, D, m, rs, ls in members:
                        s1, s2, s2b, s3, s4, s4b = scratch[gi]
                        # delta = p_l - p_r  (bf16: 2x DVE mode)
                        nc.vector.tensor_sub(s4b[:], s3[:], s4b[:])
                    for gi, D, m, rs, ls in members:
                        s1, s2, s2b, s3, s4, s4b = scratch[gi]
                        nc.vector._custom_dve(STEPD, out=D[:], in0=s4b[:],
                                              in1=D[:], s0=s0, imm2=w)

                # ---- store (the last STEPD already wrote the midpoint) ----
                for gi, D, m, rs, ls in members:
                    nc.sync.dma_start(out_v[gi], D[:])

    nc.finalize()
    return nc


_CACHED = {}


def _get_nc(n_per_core, fd=FD, group_size=GROUP):
    key = (n_per_core, fd, group_size)
    if key not in _CACHED:
        _CACHED[key] = _build_nc(n_per_core, fd, group_size)
    return _CACHED[key]


def kernel(params: np.ndarray, bid_prices: np.ndarray) -> np.ndarray:
    from concourse.bass_utils import run_bass_kernel_spmd

    params = np.ascontiguousarray(params, dtype=np.float32)
    bid_prices = np.ascontiguousarray(bid_prices, dtype=np.float32)
    n = bid_prices.shape[0]
    n_per_core = n // N_CORES

    nc = _get_nc(n_per_core)

    in_maps = []
    for i in range(N_CORES):
        sl = slice(i * n_per_core, (i + 1) * n_per_core)
        in_maps.append({"params": params[sl], "bids": bid_prices[sl]})

    res = run_bass_kernel_spmd(nc, in_maps, core_ids=list(range(N_CORES)))
    return np.concatenate([r["out"] for r in res.results], axis=0)


if __name__ == "__main__":
    rng = np.random.RandomState(0)
    n = N_TOTAL
    params = np.stack(
        [rng.randn(n).astype(np.float32),
         rng.uniform(0.2, 1.5, n).astype(np.float32)], axis=-1
    )
    bids = rng.uniform(0.1, 10.0, n).astype(np.float32)
    out = kernel(params=params, bid_prices=bids)
    print("out", out.shape, out.dtype, out[:8])


# revision 9
# speedup vs baseline: 2.3597x; 1.1187x over previous
"""Trainium2 Bass kernel: per-element maximization of the lognormal-CDF
surplus  s(d) = bid*(1-d)*Phi((ln(d*bid)-mu)/sigma),  d in [0,1].

Algorithm: the reference runs 20 golden-section iterations on s(d) (two
surplus evaluations per iteration).  s is log-concave in d (product of
log-concave factors composed with concave increasing maps), so s' crosses
zero exactly once and the argmax can instead be found by BISECTION ON THE
SIGN OF s'(d) - one evaluation per iteration, 0.5x interval shrink per
iteration (vs 0.618x for GSS):

  s'(d) >= 0  <=>  (1-d) * B * phi0 * e^{-z^2} >= d * (1 + erf z)
     z = (ln d - m) * B,  m = mu - ln bid,  B = 1/(sigma*sqrt2),
     phi0 = 2/sqrt(pi)

Validated against the reference output: rel-L2 plateaus at ~5.2e-3 for
K >= 11 (the residual is reference fp32 (1+erf) quantization noise on
deep-tail elements, not bisection resolution) - comfortably under the
2e-2 gate, and robust to 1e-3 activation-table error.

Implementation notes (per [128, FD] chunk):
  - Only the interval MIDPOINT D is tracked:  D' = D + (delta>=0)*w - w/2,
    one fused custom-DVE op (GSS_STEPD); the final STEPD emits the answer.
  - z' = (ln D - m)/sigma; erf(z) via ACT Erf with input scale 1/sqrt2;
    the gaussian side folds B*phi0 into the ACT Exp bias:
      P' = Exp(-(0.5 z'^2 + ln sigma) + ln(phi0/sqrt2)) = B*phi0*e^{-z^2}
    so the per-iter ACT tables are {Ln, Exp} (one set) + {Erf}: 2 swaps.
  - Custom fused DVE ops (registered at import): sq-scale-add for the Exp
    argument, the step update, and iteration-0 specializations (D_0 = 0.5
    is a compile-time constant, so iter 0 needs no Ln and folds M=0.5
    into the Exp bias).
  - Work is split DVE / Pool (gpsimd) / ACT; the delta & p_r ops alternate
    between DVE and Pool by chunk parity to balance the two queues.
"""
import sys

sys.path.insert(0, "/opt/trn_rl_repo")

import numpy as np

N_TOTAL = 16777216
N_CORES = 8
N_PER_CORE = N_TOTAL // N_CORES  # 2097152
P = 128
FD = 1024
GROUP = 4
K_ITERS = 8

LN_HALF = float(np.log(0.5))
INV_SQRT2 = float(1.0 / np.sqrt(2.0))
# ln(phi0/sqrt2), phi0 = 2/sqrt(pi)
LNPHI = float(np.log(2.0 / np.sqrt(np.pi)) - 0.5 * np.log(2.0))
LNPHI_HALF = float(LNPHI + np.log(0.5))  # iter-0: M = 0.5 folded in

_ops_registered = {}


def _register_ops():
    """Register the fused custom-DVE ops (documented extension point:
    dve_ops.OPS + _SUB_OPCODE_FOR_NAME + CUSTOM_DVE_SPECS). uops_sha is
    computed here the same way DveOp.compile derives it."""
    if _ops_registered:
        return _ops_registered
    import concourse.dve_ops as dve_ops
    from concourse.dve_ops import DveOp, OPS
    from concourse.dve_spec import Spec, Src0, Src1, C0, C2, Zero, sq, lower
    from concourse.dve_spec import _has_src1 as has_src1
    from concourse.dve_uop import DveOpSpec

    def ref_sqsa(in0, in1, s0, s1, imm2):
        return (np.float32(s0) * in0.astype(np.float32) ** 2 + in1).astype(np.float32)

    def ref_stepd(in0, in1, s0, s1, imm2):
        return (in1 + (in0 >= 0).astype(np.float32) * np.float32(imm2)
                + np.float32(s0)).astype(np.float32)

    def ref_z0(in0, in1, s0, s1, imm2):
        return ((np.float32(s0) - in0.astype(np.float32)) * in1).astype(np.float32)

    def ref_step0(in0, in1, s0, s1, imm2):
        return ((in0 >= 0).astype(np.float32) * np.float32(imm2)
                + np.float32(s0)).astype(np.float32)

    defs = [
        # v2 = 0.5*z'^2 + ln(sigma)
        ("GSS_SQSA", sq(Src0) * C0 + Src1, ref_sqsa),
        # D' = D + (delta >= 0)*w - w/2
        ("GSS_STEPD", Src1 + (Src0 >= Zero) * C2 + C0, ref_stepd),
        # z0 = (ln(0.5) - m) * (1/sigma)
        ("GSS_Z0", (C0 - Src0) * Src1, ref_z0),
        # D1 = (delta >= 0)*0.5 + 0.25
        ("GSS_STEP0", (Src0 >= Zero) * C2 + C0, ref_step0),
    ]
    for name, body, ref in defs:
        if name in dve_ops._SUB_OPCODE_FOR_NAME:
            _ops_registered[name] = next(o for o in OPS if o.name == name)
            continue
        row = dve_ops._CUSTOM_DVE_ROW_BASE + len(OPS)
        assert row < 0x20
        spec = Spec(body=body, reference=ref)
        shas = {}
        for ver in ("v3", "v4"):
            uops = lower(spec, ver=ver)
            shas[ver] = DveOpSpec(
                name=name, opcode=row, uops=uops, rd1_en=has_src1(spec)
            ).sha(ver)
        op = DveOp(name, spec, subdim=False, uops_sha=shas)
        OPS.append(op)
        dve_ops._SUB_OPCODE_FOR_NAME[name] = row
        dve_ops.CUSTOM_DVE_SPECS[name] = spec
        _ops_registered[name] = op
    return _ops_registered


def _build_nc(n_per_core, fd, group_size):
    import concourse.bass as bass  # noqa: F401
    import concourse.bacc as bacc
    import concourse.mybir as mybir
    import concourse.tile as tile

    ops = _register_ops()
    SQSA, STEPD, Z0, STEP0 = (
        ops["GSS_SQSA"], ops["GSS_STEPD"], ops["GSS_Z0"], ops["GSS_STEP0"]
    )

    AF = mybir.ActivationFunctionType
    ALU = mybir.AluOpType
    dt = mybir.dt.float32
    dtb = mybir.dt.bfloat16
    dth = mybir.dt.float16

    n_chunks = n_per_core // (P * fd)
    assert n_chunks * P * fd == n_per_core

    nc = bacc.Bacc(None, target_bir_lowering=False)

    def register_const(value: float):
        if (dt, value) in nc.const_aps.aps:
            return
        t = nc.alloc_sbuf_tensor(f"const-f32-c{len(nc.const_aps.aps)}", [128, 1], dt)
        nc.gpsimd.memset(t.ap(), value)
        nc.const_aps.aps[(dt, value)] = t.ap()

    for v in (0.0, LNPHI, LNPHI_HALF):
        register_const(float(v))
    nc.all_engine_barrier()

    params = nc.declare_dram_parameter("params", [n_per_core, 2], dt, isOutput=False)
    bids = nc.declare_dram_parameter("bids", [n_per_core], dt, isOutput=False)
    out = nc.declare_dram_parameter("out", [n_per_core], dt, isOutput=True)

    params_v = params.rearrange("(g p f) c -> g p (f c)", p=P, f=fd)
    bids_v = bids.rearrange("(g p f) -> g p f", p=P, f=fd)
    out_v = out.rearrange("(g p f) -> g p f", p=P, f=fd)

    with tile.TileContext(nc) as tc:
        with (
            tc.tile_pool(name="st_d", bufs=group_size + 1) as p_d,
            tc.tile_pool(name="st_m", bufs=group_size + 1) as p_m,
            tc.tile_pool(name="st_rs", bufs=group_size + 1) as p_rs,
            tc.tile_pool(name="st_ls", bufs=group_size + 1) as p_ls,
            tc.tile_pool(name="s1", bufs=group_size + 1) as p_s1,
            tc.tile_pool(name="s2", bufs=group_size + 1) as p_s2,
            tc.tile_pool(name="s3", bufs=group_size + 1) as p_s3,
            tc.tile_pool(name="s4", bufs=group_size + 1) as p_s4,
            tc.tile_pool(name="s2b", bufs=group_size + 1) as p_s2b,
            tc.tile_pool(name="s4b", bufs=group_size + 1) as p_s4b,
            tc.tile_pool(name="pload", bufs=2) as p_pl,
        ):
            for g0 in range(0, n_chunks, group_size):
                members = []
                # ---- per-chunk setup ----
                for gi in range(g0, min(g0 + group_size, n_chunks)):
                    D = p_d.tile([P, fd], dt, tag="D")
                    m = p_m.tile([P, fd], dth, tag="m")
                    rs = p_rs.tile([P, fd], dth, tag="rs")
                    ls = p_ls.tile([P, fd], dth, tag="ls")
                    # bid -> fp32 staging, then ln in place
                    pl0 = p_pl.tile([P, fd], dt, tag="plbid", bufs=2)
                    nc.sync.dma_start(pl0[:], bids_v[gi])
                    nc.scalar.activation(pl0[:], pl0[:], AF.Ln)
                    for h in range(2):
                        pl = p_pl.tile([P, fd], dt, tag="pl", bufs=2)
                        nc.sync.dma_start(pl[:], params_v[gi, :, h * fd:(h + 1) * fd])
                        plv = pl.rearrange("p (f c) -> p f c", c=2)
                        half = slice(h * (fd // 2), (h + 1) * (fd // 2))
                        # ls = ln(sigma); rs = 1/sigma; m = mu - ln(bid)
                        nc.scalar.activation(ls[:, half], plv[:, :, 1], AF.Ln)
                        rf = p_pl.tile([P, fd // 2], dt, tag="rf", bufs=2)
                        nc.vector.reciprocal_approx_fast(out=rf[:], in_=plv[:, :, 1])
                        nc.scalar.activation(rs[:, half], rf[:], AF.Copy,
                                             scale=1.0, bias=0.0)
                        nc.vector.tensor_sub(m[:, half], plv[:, :, 0],
                                             pl0[:, half])
                    members.append((gi, D, m, rs, ls))

                # ---- iteration 0: D_0 = 0.5 (compile-time constant) ----
                scratch = {}
                for gi, D, m, rs, ls in members:
                    s1 = p_s1.tile([P, fd], dth, tag="s1")
                    s2 = p_s2.tile([P, fd], dth, tag="s2")
                    s3 = p_s3.tile([P, fd], dt, tag="s3i0")
                    s4 = p_s4.tile([P, fd], dt, tag="s4")
                    scratch[gi] = (s1, s2, s3, s4)
                    nc.vector._custom_dve(Z0, out=s1[:], in0=m[:], in1=rs[:],
                                          s0=LN_HALF)
                for gi, D, m, rs, ls in members:
                    s1, s2, s3, s4 = scratch[gi]
                    nc.scalar.activation(s4[:], s1[:], AF.Erf, scale=INV_SQRT2)
                for gi, D, m, rs, ls in members:
                    s1, s2, s3, s4 = scratch[gi]
                    nc.vector._custom_dve(SQSA, out=s2[:], in0=s1[:], in1=ls[:],
                                          s0=0.5)
                for gi, D, m, rs, ls in members:
                    s1, s2, s3, s4 = scratch[gi]
                    # p_l = P' * 0.5 (M folded into bias)
                    nc.scalar.activation(s3[:], s2[:], AF.Exp, scale=-1.0,
                                         bias=LNPHI_HALF)
                for gi, D, m, rs, ls in members:
                    s1, s2, s3, s4 = scratch[gi]
                    # p_r = (E+1)*0.5
                    nc.vector.tensor_scalar(s4[:], s4[:], 1.0, 0.5,
                                            op0=ALU.add, op1=ALU.mult)
                for gi, D, m, rs, ls in members:
                    s1, s2, s3, s4 = scratch[gi]
                    nc.vector.tensor_sub(s4[:], s3[:], s4[:])
                for gi, D, m, rs, ls in members:
                    s1, s2, s3, s4 = scratch[gi]
                    nc.vector._custom_dve(STEP0, out=D[:], in0=s4[:],
                                          s0=0.25, imm2=0.5)

                # ---- iterations 1..K-1 ----
                for k in range(1, K_ITERS):
                    w = float(2.0 ** -(k + 1))
                    s0 = float(-(2.0 ** -(k + 2)))
                    for gi, D, m, rs, ls in members:
                        s1 = p_s1.tile([P, fd], dth, tag="s1")
                        s2 = p_s2.tile([P, fd], dth, tag="s2")
                        s2b = p_s2b.tile([P, fd], dtb, tag="s2b")
                        s3 = p_s3.tile([P, fd], dtb, tag="s3")
                        s4 = p_s4.tile([P, fd], dt, tag="s4")
                        s4b = p_s4b.tile([P, fd], dtb, tag="s4b")
                        scratch[gi] = (s1, s2, s2b, s3, s4, s4b)
                        nc.scalar.activation(s1[:], D[:], AF.Ln)
                    for gi, D, m, rs, ls in members:
                        s1, s2, s2b, s3, s4, s4b = scratch[gi]
                        # M = 1 - D  (Copy is in every ACT table set), bf16
                        nc.scalar.activation(s3[:], D[:], AF.Copy, scale=-1.0,
                                             bias=1.0)
                    for gi, D, m, rs, ls in members:
                        s1, s2, s2b, s3, s4, s4b = scratch[gi]
                        nc.vector.tensor_sub(s1[:], s1[:], m[:])
                    for gi, D, m, rs, ls in members:
                        s1, s2, s2b, s3, s4, s4b = scratch[gi]
                        nc.vector.tensor_mul(s1[:], s1[:], rs[:])
                    for gi, D, m, rs, ls in members:
                        s1, s2, s2b, s3, s4, s4b = scratch[gi]
                        nc.scalar.activation(s4[:], s1[:], AF.Erf, scale=INV_SQRT2)
                    for gi, D, m, rs, ls in members:
                        s1, s2, s2b, s3, s4, s4b = scratch[gi]
                        nc.vector._custom_dve(SQSA, out=s2[:], in0=s1[:],
                                              in1=ls[:], s0=0.5)
                    for gi, D, m, rs, ls in members:
                        s1, s2, s2b, s3, s4, s4b = scratch[gi]
                        # P' in bf16
                        nc.scalar.activation(s2b[:], s2[:], AF.Exp, scale=-1.0,
                                             bias=LNPHI)
                    for gi, D, m, rs, ls in members:
                        s1, s2, s2b, s3, s4, s4b = scratch[gi]
                        # p_l = P' * M  (bf16 x bf16 -> bf16: 2x DVE mode)
                        nc.vector.tensor_mul(s3[:], s2b[:], s3[:])
                    for gi, D, m, rs, ls in members:
                        s1, s2, s2b, s3, s4, s4b = scratch[gi]
                        # p_r = (E+1)*D -> bf16
                        nc.vector.scalar_tensor_tensor(s4b[:], s4[:], 1.0, D[:],
                                                       op0=ALU.add, op1=ALU.mult)
                    for gi, D, m, rs, ls in members:
                        s1, s2, s2b, s3, s4, s4b = scratch[gi]
                        # delta = p_l - p_r  (bf16: 2x DVE mode)
                        nc.vector.tensor_sub(s4b[:], s3[:], s4b[:])
                    for gi, D, m, rs, ls in members:
                        s1, s2, s2b, s3, s4, s4b = scratch[gi]
                        nc.vector._custom_dve(STEPD, out=D[:], in0=s4b[:],
                                              in1=D[:], s0=s0, imm2=w)

                # ---- store (the last STEPD already wrote the midpoint) ----
                for gi, D, m, rs, ls in members:
                    nc.sync.dma_start(out_v[gi], D[:])

    nc.finalize()
    return nc


_CACHED = {}


def _get_nc(n_per_core, fd=FD, group_size=GROUP):
    key = (n_per_core, fd, group_size)
    if key not in _CACHED:
        _CACHED[key] = _build_nc(n_per_core, fd, group_size)
    return _CACHED[key]


def kernel(params: np.ndarray, bid_prices: np.ndarray) -> np.ndarray:
    from concourse.bass_utils import run_bass_kernel_spmd

    params = np.ascontiguousarray(params, dtype=np.float32)
    bid_prices = np.ascontiguousarray(bid_prices, dtype=np.float32)
    n = bid_prices.shape[0]
    n_per_core = n // N_CORES

    nc = _get_nc(n_per_core)

    in_maps = []
    for i in range(N_CORES):
        sl = slice(i * n_per_core, (i + 1) * n_per_core)
        in_maps.append({"params": params[sl], "bids": bid_prices[sl]})

    res = run_bass_kernel_spmd(nc, in_maps, core_ids=list(range(N_CORES)))
    return np.concatenate([r["out"] for r in res.results], axis=0)


if __name__ == "__main__":
    rng = np.random.RandomState(0)
    n = N_TOTAL
    params = np.stack(
        [rng.randn(n).astype(np.float32),
         rng.uniform(0.2, 1.5, n).astype(np.float32)], axis=-1
    )
    bids = rng.uniform(0.1, 10.0, n).astype(np.float32)
    out = kernel(params=params, bid_prices=bids)
    print("out", out.shape, out.dtype, out[:8])


# revision 10
# speedup vs baseline: 2.4045x; 1.0190x over previous
"""Trainium2 Bass kernel: per-element maximization of the lognormal-CDF
surplus  s(d) = bid*(1-d)*Phi((ln(d*bid)-mu)/sigma),  d in [0,1].

Algorithm: the reference runs 20 golden-section iterations on s(d) (two
surplus evaluations per iteration).  s is log-concave in d (product of
log-concave factors composed with concave increasing maps), so s' crosses
zero exactly once and the argmax is instead found by BISECTION ON THE
SIGN OF s'(d) - one evaluation per iteration, 0.5x interval shrink
(vs 0.618x for GSS), K=8 iterations:

  s'(d) >= 0  <=>  (1-d) * B * phi0 * e^{-z^2} >= d * (1 + erf z)
     z = (ln d - m) * B,  m = mu - ln bid,  B = 1/(sigma*sqrt2),
     phi0 = 2/sqrt(pi)

evaluated in LOG space (both sides > 0):

  ln(1-d) - ln(1+erf z) - ln d - 0.5 z'^2 - ln(sigma) + ln(phi0/sqrt2) >= 0
     with z' = (ln d - m)/sigma = sqrt2 * z

which needs only {Ln, Erf} activation tables (2 table swaps/iteration)
and no Exp.  Saturation is exact: erf -> -1 gives LG = Ln(0) = -inf so
delta = +inf -> step right, matching the reference's tie behavior.

Validated against the reference output in simulation (rel-L2 5.8e-3;
the plateau is reference fp32 (1+erf) quantization noise on deep-tail
elements, not bisection resolution); measured on HW at ~3e-3 - well
under the 2e-2 gate.  Robust to 1e-3 activation-table error.

Implementation (per [128, FD] chunk):
  - Only the interval MIDPOINT D is tracked:  D' = D + (delta>=c)*w - w/2,
    one fused custom-DVE op; the final step emits the answer directly.
  - fp16 intermediates: the five chained subtractions/multiplies run as
    2-byte packed TENSOR_TENSOR ops in the DVE 2x_1p mode (2 elem/cycle);
    E stays fp32 so (1+erf) keeps fp32 resolution near saturation; D
    stays fp32 (needs 2^-K resolution near 1).
  - Custom fused DVE ops (registered at import): GSS_SQSB computes
    0.5*z'^2 + ln(sigma) - ln(phi0/sqrt2) in one pass; GSS_STEPD2 does
    compare+step+recenter in one pass; GSS_Z0/GSS_STEP0LE specialize
    iteration 0 (D_0 = 0.5 is a compile-time constant: no Ln needed, and
    ln(D_0) = ln(1-D_0) cancels in the condition).
  - GpSimd (Pool) is NOT used for elementwise work: it shares SBUF ports
    with the DVE and concurrent pool/DVE streams slow each other ~2.8x
    (measured); everything runs on DVE + ACT.
"""
import sys

sys.path.insert(0, "/opt/trn_rl_repo")

import numpy as np

N_TOTAL = 16777216
N_CORES = 8
N_PER_CORE = N_TOTAL // N_CORES  # 2097152
P = 128
FD = 1024
GROUP = 4
K_ITERS = 8

LN_HALF = float(np.log(0.5))
INV_SQRT2 = float(1.0 / np.sqrt(2.0))
# ln(phi0/sqrt2), phi0 = 2/sqrt(pi)
LNPHI = float(np.log(2.0 / np.sqrt(np.pi)) - 0.5 * np.log(2.0))

_ops_registered = {}


def _register_ops():
    """Register the fused custom-DVE ops (documented extension point:
    dve_ops.OPS + _SUB_OPCODE_FOR_NAME + CUSTOM_DVE_SPECS). uops_sha is
    computed here the same way DveOp.compile derives it."""
    if _ops_registered:
        return _ops_registered
    import concourse.dve_ops as dve_ops
    from concourse.dve_ops import DveOp, OPS
    from concourse.dve_spec import Spec, Src0, Src1, C0, C1, C2, sq, lower
    from concourse.dve_spec import _has_src1 as has_src1
    from concourse.dve_uop import DveOpSpec

    def ref_sqsb(in0, in1, s0, s1, imm2):
        return (np.float32(s0) * in0.astype(np.float32) ** 2 + in1
                + np.float32(s1)).astype(np.float32)

    def ref_stepd(in0, in1, s0, s1, imm2):
        return (in1 + (in0 >= np.float32(s1)).astype(np.float32)
                * np.float32(imm2) + np.float32(s0)).astype(np.float32)

    def ref_z0(in0, in1, s0, s1, imm2):
        return ((np.float32(s0) - in0.astype(np.float32)) * in1).astype(np.float32)

    def ref_step0le(in0, in1, s0, s1, imm2):
        return ((np.float32(s1) >= in0).astype(np.float32) * np.float32(imm2)
                + np.float32(s0)).astype(np.float32)

    defs = [
        # v = 0.5*z'^2 + ln(sigma) + (-ln(phi0/sqrt2))
        ("GSS_SQSB", sq(Src0) * C0 + Src1 + C1, ref_sqsb),
        # D' = D + (delta >= thresh)*w - w/2
        ("GSS_STEPD2", Src1 + (Src0 >= C1) * C2 + C0, ref_stepd),
        # z0 = (ln(0.5) - m) * (1/sigma)
        ("GSS_Z0", (C0 - Src0) * Src1, ref_z0),
        # D1 = (thresh >= q)*0.5 + 0.25
        ("GSS_STEP0LE", (C1 >= Src0) * C2 + C0, ref_step0le),
    ]
    for name, body, ref in defs:
        if name in dve_ops._SUB_OPCODE_FOR_NAME:
            _ops_registered[name] = next(o for o in OPS if o.name == name)
            continue
        row = dve_ops._CUSTOM_DVE_ROW_BASE + len(OPS)
        assert row < 0x20
        spec = Spec(body=body, reference=ref)
        shas = {}
        for ver in ("v3", "v4"):
            uops = lower(spec, ver=ver)
            shas[ver] = DveOpSpec(
                name=name, opcode=row, uops=uops, rd1_en=has_src1(spec)
            ).sha(ver)
        op = DveOp(name, spec, subdim=False, uops_sha=shas)
        OPS.append(op)
        dve_ops._SUB_OPCODE_FOR_NAME[name] = row
        dve_ops.CUSTOM_DVE_SPECS[name] = spec
        _ops_registered[name] = op
    return _ops_registered


def _build_nc(n_per_core, fd, group_size):
    import concourse.bass as bass  # noqa: F401
    import concourse.bacc as bacc
    import concourse.mybir as mybir
    import concourse.tile as tile

    ops = _register_ops()
    SQSB, STEPD2, Z0, STEP0LE = (
        ops["GSS_SQSB"], ops["GSS_STEPD2"], ops["GSS_Z0"], ops["GSS_STEP0LE"]
    )

    AF = mybir.ActivationFunctionType
    dt = mybir.dt.float32
    dth = mybir.dt.float16

    n_chunks = n_per_core // (P * fd)
    assert n_chunks * P * fd == n_per_core

    nc = bacc.Bacc(None, target_bir_lowering=False)

    def register_const(value: float):
        if (dt, value) in nc.const_aps.aps:
            return
        t = nc.alloc_sbuf_tensor(f"const-f32-c{len(nc.const_aps.aps)}", [128, 1], dt)
        nc.gpsimd.memset(t.ap(), value)
        nc.const_aps.aps[(dt, value)] = t.ap()

    for v in (0.0, 1.0):
        register_const(float(v))
    nc.all_engine_barrier()

    params = nc.declare_dram_parameter("params", [n_per_core, 2], dt, isOutput=False)
    bids = nc.declare_dram_parameter("bids", [n_per_core], dt, isOutput=False)
    out = nc.declare_dram_parameter("out", [n_per_core], dt, isOutput=True)

    params_v = params.rearrange("(g p f) c -> g p (f c)", p=P, f=fd)
    bids_v = bids.rearrange("(g p f) -> g p f", p=P, f=fd)
    out_v = out.rearrange("(g p f) -> g p f", p=P, f=fd)

    with tile.TileContext(nc) as tc:
        with (
            tc.tile_pool(name="st_d", bufs=group_size + 1) as p_d,
            tc.tile_pool(name="st_m", bufs=group_size + 1) as p_m,
            tc.tile_pool(name="st_rs", bufs=group_size + 1) as p_rs,
            tc.tile_pool(name="st_ls", bufs=group_size + 1) as p_ls,
            tc.tile_pool(name="sL", bufs=group_size + 1) as p_sL,
            tc.tile_pool(name="s1", bufs=group_size + 1) as p_s1,
            tc.tile_pool(name="sLM", bufs=group_size + 1) as p_sLM,
            tc.tile_pool(name="sE", bufs=group_size + 1) as p_sE,
            tc.tile_pool(name="sLG", bufs=group_size + 1) as p_sLG,
            tc.tile_pool(name="s2", bufs=group_size + 1) as p_s2,
            tc.tile_pool(name="pload", bufs=2) as p_pl,
        ):
            for g0 in range(0, n_chunks, group_size):
                members = []
                # ---- per-chunk setup ----
                for gi in range(g0, min(g0 + group_size, n_chunks)):
                    D = p_d.tile([P, fd], dt, tag="D")
                    m = p_m.tile([P, fd], dth, tag="m")
                    rs = p_rs.tile([P, fd], dth, tag="rs")
                    ls = p_ls.tile([P, fd], dth, tag="ls")
                    # bid -> fp32 staging, then ln in place
                    pl0 = p_pl.tile([P, fd], dt, tag="plbid", bufs=2)
                    nc.sync.dma_start(pl0[:], bids_v[gi])
                    nc.scalar.activation(pl0[:], pl0[:], AF.Ln)
                    for h in range(2):
                        pl = p_pl.tile([P, fd], dt, tag="pl", bufs=2)
                        nc.sync.dma_start(pl[:], params_v[gi, :, h * fd:(h + 1) * fd])
                        plv = pl.rearrange("p (f c) -> p f c", c=2)
                        half = slice(h * (fd // 2), (h + 1) * (fd // 2))
                        # ls = ln(sigma); rs = 1/sigma; m = mu - ln(bid)
                        nc.scalar.activation(ls[:, half], plv[:, :, 1], AF.Ln)
                        rf = p_pl.tile([P, fd // 2], dt, tag="rf", bufs=2)
                        nc.vector.reciprocal_approx_fast(out=rf[:], in_=plv[:, :, 1])
                        nc.scalar.activation(rs[:, half], rf[:], AF.Copy,
                                             scale=1.0, bias=0.0)
                        nc.vector.tensor_sub(m[:, half], plv[:, :, 0], pl0[:, half])
                    members.append((gi, D, m, rs, ls))

                # ---- iteration 0: D_0 = 0.5 (compile-time constant) ----
                # ln(1-D0) and ln(D0) cancel: condition is  LG + v0 <= 0.
                scratch = {}
                for gi, D, m, rs, ls in members:
                    s1 = p_s1.tile([P, fd], dth, tag="s1")
                    sE = p_sE.tile([P, fd], dt, tag="sE")
                    sLG = p_sLG.tile([P, fd], dth, tag="sLG")
                    s2 = p_s2.tile([P, fd], dth, tag="s2")
                    scratch[gi] = (s1, sE, sLG, s2)
                    nc.vector._custom_dve(Z0, out=s1[:], in0=m[:], in1=rs[:],
                                          s0=LN_HALF)
                for gi, D, m, rs, ls in members:
                    s1, sE, sLG, s2 = scratch[gi]
                    nc.scalar.activation(sE[:], s1[:], AF.Erf, scale=INV_SQRT2)
                for gi, D, m, rs, ls in members:
                    s1, sE, sLG, s2 = scratch[gi]
                    nc.vector._custom_dve(SQSB, out=s2[:], in0=s1[:], in1=ls[:],
                                          s0=0.5, s1=-LNPHI)
                for gi, D, m, rs, ls in members:
                    s1, sE, sLG, s2 = scratch[gi]
                    nc.scalar.activation(sLG[:], sE[:], AF.Ln, bias=1.0)
                for gi, D, m, rs, ls in members:
                    s1, sE, sLG, s2 = scratch[gi]
                    # q = LG + v0  (fp16 2x)
                    nc.vector.tensor_add(s2[:], sLG[:], s2[:])
                for gi, D, m, rs, ls in members:
                    s1, sE, sLG, s2 = scratch[gi]
                    nc.vector._custom_dve(STEP0LE, out=D[:], in0=s2[:],
                                          s0=0.25, s1=0.0, imm2=0.5)

                # ---- iterations 1..K-1 (log-space derivative sign) ----
                for k in range(1, K_ITERS):
                    w = float(2.0 ** -(k + 1))
                    s0c = float(-(2.0 ** -(k + 2)))
                    for gi, D, m, rs, ls in members:
                        sL = p_sL.tile([P, fd], dth, tag="sL")
                        s1 = p_s1.tile([P, fd], dth, tag="s1")
                        sLM = p_sLM.tile([P, fd], dth, tag="sLM")
                        sE = p_sE.tile([P, fd], dt, tag="sE")
                        sLG = p_sLG.tile([P, fd], dth, tag="sLG")
                        s2 = p_s2.tile([P, fd], dth, tag="s2")
                        scratch[gi] = (sL, s1, sLM, sE, sLG, s2)
                        nc.scalar.activation(sL[:], D[:], AF.Ln)
                    for gi, D, m, rs, ls in members:
                        sL, s1, sLM, sE, sLG, s2 = scratch[gi]
                        # LM = ln(1 - D)
                        nc.scalar.activation(sLM[:], D[:], AF.Ln, scale=-1.0,
                                             bias=1.0)
                    for gi, D, m, rs, ls in members:
                        sL, s1, sLM, sE, sLG, s2 = scratch[gi]
                        # t = L - m  (fp16 2x)
                        nc.vector.tensor_sub(s1[:], sL[:], m[:])
                    for gi, D, m, rs, ls in members:
                        sL, s1, sLM, sE, sLG, s2 = scratch[gi]
                        # z' = t / sigma  (fp16 2x)
                        nc.vector.tensor_mul(s1[:], s1[:], rs[:])
                    for gi, D, m, rs, ls in members:
                        sL, s1, sLM, sE, sLG, s2 = scratch[gi]
                        nc.scalar.activation(sE[:], s1[:], AF.Erf, scale=INV_SQRT2)
                    for gi, D, m, rs, ls in members:
                        sL, s1, sLM, sE, sLG, s2 = scratch[gi]
                        # v = 0.5 z'^2 + ln(sigma) - ln(phi0/sqrt2)
                        nc.vector._custom_dve(SQSB, out=s2[:], in0=s1[:],
                                              in1=ls[:], s0=0.5, s1=-LNPHI)
                    for gi, D, m, rs, ls in members:
                        sL, s1, sLM, sE, sLG, s2 = scratch[gi]
                        # LG = ln(1 + erf z)  (-inf at saturation: step right)
                        nc.scalar.activation(sLG[:], sE[:], AF.Ln, bias=1.0)
                    for gi, D, m, rs, ls in members:
                        sL, s1, sLM, sE, sLG, s2 = scratch[gi]
                        # a = LM - LG  (fp16 2x, in place)
                        nc.vector.tensor_sub(sLM[:], sLM[:], sLG[:])
                    for gi, D, m, rs, ls in members:
                        sL, s1, sLM, sE, sLG, s2 = scratch[gi]
                        # b = a - L  (fp16 2x, in place)
                        nc.vector.tensor_sub(sLM[:], sLM[:], sL[:])
                    for gi, D, m, rs, ls in members:
                        sL, s1, sLM, sE, sLG, s2 = scratch[gi]
                        # delta = b - v  (fp16 2x, in place)
                        nc.vector.tensor_sub(sLM[:], sLM[:], s2[:])
                    for gi, D, m, rs, ls in members:
                        sL, s1, sLM, sE, sLG, s2 = scratch[gi]
                        nc.vector._custom_dve(STEPD2, out=D[:], in0=sLM[:],
                                              in1=D[:], s0=s0c, s1=0.0, imm2=w)

                # ---- store (the last step already wrote the midpoint) ----
                for gi, D, m, rs, ls in members:
                    nc.sync.dma_start(out_v[gi], D[:])

    nc.finalize()
    return nc


_CACHED = {}


def _get_nc(n_per_core, fd=FD, group_size=GROUP):
    key = (n_per_core, fd, group_size)
    if key not in _CACHED:
        _CACHED[key] = _build_nc(n_per_core, fd, group_size)
    return _CACHED[key]


def kernel(params: np.ndarray, bid_prices: np.ndarray) -> np.ndarray:
    from concourse.bass_utils import run_bass_kernel_spmd

    params = np.ascontiguousarray(params, dtype=np.float32)
    bid_prices = np.ascontiguousarray(bid_prices, dtype=np.float32)
    n = bid_prices.shape[0]
    n_per_core = n // N_CORES

    nc = _get_nc(n_per_core)

    in_maps = []
    for i in range(N_CORES):
        sl = slice(i * n_per_core, (i + 1) * n_per_core)
        in_maps.append({"params": params[sl], "bids": bid_prices[sl]})

    res = run_bass_kernel_spmd(nc, in_maps, core_ids=list(range(N_CORES)))
    return np.concatenate([r["out"] for r in res.results], axis=0)


if __name__ == "__main__":
    rng = np.random.RandomState(0)
    n = N_TOTAL
    params = np.stack(
        [rng.randn(n).astype(np.float32),
         rng.uniform(0.2, 1.5, n).astype(np.float32)], axis=-1
    )
    bids = rng.uniform(0.1, 10.0, n).astype(np.float32)
    out = kernel(params=params, bid_prices=bids)
    print("out", out.shape, out.dtype, out[:8])


# revision 11
# speedup vs baseline: 2.7066x; 1.1256x over previous
"""Trainium2 Bass kernel: per-element maximization of the lognormal-CDF
surplus  s(d) = bid*(1-d)*Phi((ln(d*bid)-mu)/sigma),  d in [0,1].

Algorithm: the reference runs 20 golden-section iterations on s(d) (two
surplus evaluations per iteration).  s is log-concave in d (product of
log-concave factors composed with concave increasing maps), so s' crosses
zero exactly once and the argmax is instead found by BISECTION ON THE
SIGN OF s'(d) - one evaluation per iteration, 0.5x interval shrink
(vs 0.618x for GSS), K=7 iterations:

  s'(d) >= 0  <=>  (1-d) * B * phi0 * e^{-z^2} >= d * (1 + erf z)
     z = (ln d - m) * B,  m = mu - ln bid,  B = 1/(sigma*sqrt2),
     phi0 = 2/sqrt(pi)

evaluated in LOG space (both sides > 0):

  ln(1-d) - ln(1+erf z) - ln d - 0.5 z'^2 - ln(sigma) + ln(phi0/sqrt2) >= 0
     with z' = (ln d - m)/sigma = sqrt2 * z

which needs only {Ln, Erf} activation tables (2 table swaps/iteration)
and no Exp.  Saturation is exact: erf -> -1 gives LG = Ln(0) = -inf so
delta = +inf -> step right, matching the reference's tie behavior.

Validated against the reference output in simulation (rel-L2 5.8e-3;
the plateau is reference fp32 (1+erf) quantization noise on deep-tail
elements, not bisection resolution); measured on HW at ~3e-3 - well
under the 2e-2 gate.  Robust to 1e-3 activation-table error.

Implementation (per [128, FD] chunk):
  - Only the interval MIDPOINT D is tracked:  D' = D + (delta>=c)*w - w/2,
    one fused custom-DVE op; the final step emits the answer directly.
  - fp16 intermediates: the five chained subtractions/multiplies run as
    2-byte packed TENSOR_TENSOR ops in the DVE 2x_1p mode (2 elem/cycle);
    E stays fp32 so (1+erf) keeps fp32 resolution near saturation; D
    stays fp32 (needs 2^-K resolution near 1).
  - Custom fused DVE ops (registered at import): GSS_SQSB computes
    0.5*z'^2 + ln(sigma) - ln(phi0/sqrt2) in one pass; GSS_STEPD2 does
    compare+step+recenter in one pass; GSS_Z0/GSS_STEP0LE specialize
    iteration 0 (D_0 = 0.5 is a compile-time constant: no Ln needed, and
    ln(D_0) = ln(1-D_0) cancels in the condition).
  - GpSimd (Pool) is NOT used for elementwise work: it shares SBUF ports
    with the DVE and concurrent pool/DVE streams slow each other ~2.8x
    (measured); everything runs on DVE + ACT.
"""
import sys

sys.path.insert(0, "/opt/trn_rl_repo")

import numpy as np

N_TOTAL = 16777216
N_CORES = 8
N_PER_CORE = N_TOTAL // N_CORES  # 2097152
P = 128
FD = 1024
GROUP = 4
K_ITERS = 7

LN_HALF = float(np.log(0.5))
INV_SQRT2 = float(1.0 / np.sqrt(2.0))
# ln(phi0/sqrt2), phi0 = 2/sqrt(pi)
LNPHI = float(np.log(2.0 / np.sqrt(np.pi)) - 0.5 * np.log(2.0))

_ops_registered = {}


def _register_ops():
    """Register the fused custom-DVE ops (documented extension point:
    dve_ops.OPS + _SUB_OPCODE_FOR_NAME + CUSTOM_DVE_SPECS). uops_sha is
    computed here the same way DveOp.compile derives it."""
    if _ops_registered:
        return _ops_registered
    import concourse.dve_ops as dve_ops
    from concourse.dve_ops import DveOp, OPS
    from concourse.dve_spec import Spec, Src0, Src1, C0, C1, C2, sq, lower
    from concourse.dve_spec import _has_src1 as has_src1
    from concourse.dve_uop import DveOpSpec

    def ref_sqsb(in0, in1, s0, s1, imm2):
        return (np.float32(s0) * in0.astype(np.float32) ** 2 + in1
                + np.float32(s1)).astype(np.float32)

    def ref_stepd(in0, in1, s0, s1, imm2):
        return (in1 + (in0 >= np.float32(s1)).astype(np.float32)
                * np.float32(imm2) + np.float32(s0)).astype(np.float32)

    def ref_z0(in0, in1, s0, s1, imm2):
        return ((np.float32(s0) - in0.astype(np.float32)) * in1).astype(np.float32)

    def ref_step0le(in0, in1, s0, s1, imm2):
        return ((np.float32(s1) >= in0).astype(np.float32) * np.float32(imm2)
                + np.float32(s0)).astype(np.float32)

    defs = [
        # v = 0.5*z'^2 + ln(sigma) + (-ln(phi0/sqrt2))
        ("GSS_SQSB", sq(Src0) * C0 + Src1 + C1, ref_sqsb),
        # D' = D + (delta >= thresh)*w - w/2
        ("GSS_STEPD2", Src1 + (Src0 >= C1) * C2 + C0, ref_stepd),
        # z0 = (ln(0.5) - m) * (1/sigma)
        ("GSS_Z0", (C0 - Src0) * Src1, ref_z0),
        # D1 = (thresh >= q)*0.5 + 0.25
        ("GSS_STEP0LE", (C1 >= Src0) * C2 + C0, ref_step0le),
    ]
    for name, body, ref in defs:
        if name in dve_ops._SUB_OPCODE_FOR_NAME:
            _ops_registered[name] = next(o for o in OPS if o.name == name)
            continue
        row = dve_ops._CUSTOM_DVE_ROW_BASE + len(OPS)
        assert row < 0x20
        spec = Spec(body=body, reference=ref)
        shas = {}
        for ver in ("v3", "v4"):
            uops = lower(spec, ver=ver)
            shas[ver] = DveOpSpec(
                name=name, opcode=row, uops=uops, rd1_en=has_src1(spec)
            ).sha(ver)
        op = DveOp(name, spec, subdim=False, uops_sha=shas)
        OPS.append(op)
        dve_ops._SUB_OPCODE_FOR_NAME[name] = row
        dve_ops.CUSTOM_DVE_SPECS[name] = spec
        _ops_registered[name] = op
    return _ops_registered


def _build_nc(n_per_core, fd, group_size):
    import concourse.bass as bass  # noqa: F401
    import concourse.bacc as bacc
    import concourse.mybir as mybir
    import concourse.tile as tile

    ops = _register_ops()
    SQSB, STEPD2, Z0, STEP0LE = (
        ops["GSS_SQSB"], ops["GSS_STEPD2"], ops["GSS_Z0"], ops["GSS_STEP0LE"]
    )

    AF = mybir.ActivationFunctionType
    dt = mybir.dt.float32
    dth = mybir.dt.float16

    n_chunks = n_per_core // (P * fd)
    assert n_chunks * P * fd == n_per_core

    nc = bacc.Bacc(None, target_bir_lowering=False)

    def register_const(value: float):
        if (dt, value) in nc.const_aps.aps:
            return
        t = nc.alloc_sbuf_tensor(f"const-f32-c{len(nc.const_aps.aps)}", [128, 1], dt)
        nc.gpsimd.memset(t.ap(), value)
        nc.const_aps.aps[(dt, value)] = t.ap()

    for v in (0.0, 1.0):
        register_const(float(v))
    nc.all_engine_barrier()

    params = nc.declare_dram_parameter("params", [n_per_core, 2], dt, isOutput=False)
    bids = nc.declare_dram_parameter("bids", [n_per_core], dt, isOutput=False)
    out = nc.declare_dram_parameter("out", [n_per_core], dt, isOutput=True)

    params_v = params.rearrange("(g p f) c -> g p (f c)", p=P, f=fd)
    bids_v = bids.rearrange("(g p f) -> g p f", p=P, f=fd)
    out_v = out.rearrange("(g p f) -> g p f", p=P, f=fd)

    with tile.TileContext(nc) as tc:
        with (
            tc.tile_pool(name="st_d", bufs=group_size + 1) as p_d,
            tc.tile_pool(name="st_m", bufs=group_size + 1) as p_m,
            tc.tile_pool(name="st_rs", bufs=group_size + 1) as p_rs,
            tc.tile_pool(name="st_ls", bufs=group_size + 1) as p_ls,
            tc.tile_pool(name="sL", bufs=group_size + 1) as p_sL,
            tc.tile_pool(name="s1", bufs=group_size + 1) as p_s1,
            tc.tile_pool(name="sLM", bufs=group_size + 1) as p_sLM,
            tc.tile_pool(name="sE", bufs=group_size + 1) as p_sE,
            tc.tile_pool(name="sLG", bufs=group_size + 1) as p_sLG,
            tc.tile_pool(name="s2", bufs=group_size + 1) as p_s2,
            tc.tile_pool(name="pload", bufs=2) as p_pl,
        ):
            for g0 in range(0, n_chunks, group_size):
                members = []
                # ---- per-chunk setup ----
                for gi in range(g0, min(g0 + group_size, n_chunks)):
                    D = p_d.tile([P, fd], dt, tag="D")
                    m = p_m.tile([P, fd], dth, tag="m")
                    rs = p_rs.tile([P, fd], dth, tag="rs")
                    ls = p_ls.tile([P, fd], dth, tag="ls")
                    # bid -> fp32 staging, then ln in place
                    pl0 = p_pl.tile([P, fd], dt, tag="plbid", bufs=2)
                    nc.sync.dma_start(pl0[:], bids_v[gi])
                    nc.scalar.activation(pl0[:], pl0[:], AF.Ln)
                    for h in range(2):
                        pl = p_pl.tile([P, fd], dt, tag="pl", bufs=2)
                        nc.sync.dma_start(pl[:], params_v[gi, :, h * fd:(h + 1) * fd])
                        plv = pl.rearrange("p (f c) -> p f c", c=2)
                        half = slice(h * (fd // 2), (h + 1) * (fd // 2))
                        # ls = ln(sigma); rs = 1/sigma; m = mu - ln(bid)
                        nc.scalar.activation(ls[:, half], plv[:, :, 1], AF.Ln)
                        rf = p_pl.tile([P, fd // 2], dt, tag="rf", bufs=2)
                        nc.vector.reciprocal_approx_fast(out=rf[:], in_=plv[:, :, 1])
                        nc.scalar.activation(rs[:, half], rf[:], AF.Copy,
                                             scale=1.0, bias=0.0)
                        nc.vector.tensor_sub(m[:, half], plv[:, :, 0], pl0[:, half])
                    members.append((gi, D, m, rs, ls))

                # ---- iteration 0: D_0 = 0.5 (compile-time constant) ----
                # ln(1-D0) and ln(D0) cancel: condition is  LG + v0 <= 0.
                scratch = {}
                for gi, D, m, rs, ls in members:
                    s1 = p_s1.tile([P, fd], dth, tag="s1")
                    sE = p_sE.tile([P, fd], dt, tag="sE")
                    sLG = p_sLG.tile([P, fd], dth, tag="sLG")
                    s2 = p_s2.tile([P, fd], dth, tag="s2")
                    scratch[gi] = (s1, sE, sLG, s2)
                    nc.vector._custom_dve(Z0, out=s1[:], in0=m[:], in1=rs[:],
                                          s0=LN_HALF)
                for gi, D, m, rs, ls in members:
                    s1, sE, sLG, s2 = scratch[gi]
                    nc.scalar.activation(sE[:], s1[:], AF.Erf, scale=INV_SQRT2)
                for gi, D, m, rs, ls in members:
                    s1, sE, sLG, s2 = scratch[gi]
                    nc.vector._custom_dve(SQSB, out=s2[:], in0=s1[:], in1=ls[:],
                                          s0=0.5, s1=-LNPHI)
                for gi, D, m, rs, ls in members:
                    s1, sE, sLG, s2 = scratch[gi]
                    nc.scalar.activation(sLG[:], sE[:], AF.Ln, bias=1.0)
                for gi, D, m, rs, ls in members:
                    s1, sE, sLG, s2 = scratch[gi]
                    # q = LG + v0  (fp16 2x)
                    nc.vector.tensor_add(s2[:], sLG[:], s2[:])
                for gi, D, m, rs, ls in members:
                    s1, sE, sLG, s2 = scratch[gi]
                    nc.vector._custom_dve(STEP0LE, out=D[:], in0=s2[:],
                                          s0=0.25, s1=0.0, imm2=0.5)

                # ---- iterations 1..K-1 (log-space derivative sign) ----
                for k in range(1, K_ITERS):
                    w = float(2.0 ** -(k + 1))
                    s0c = float(-(2.0 ** -(k + 2)))
                    for gi, D, m, rs, ls in members:
                        sL = p_sL.tile([P, fd], dth, tag="sL")
                        s1 = p_s1.tile([P, fd], dth, tag="s1")
                        sLM = p_sLM.tile([P, fd], dth, tag="sLM")
                        sE = p_sE.tile([P, fd], dt, tag="sE")
                        sLG = p_sLG.tile([P, fd], dth, tag="sLG")
                        s2 = p_s2.tile([P, fd], dth, tag="s2")
                        scratch[gi] = (sL, s1, sLM, sE, sLG, s2)
                        nc.scalar.activation(sL[:], D[:], AF.Ln)
                    for gi, D, m, rs, ls in members:
                        sL, s1, sLM, sE, sLG, s2 = scratch[gi]
                        # LM = ln(1 - D)
                        nc.scalar.activation(sLM[:], D[:], AF.Ln, scale=-1.0,
                                             bias=1.0)
                    for gi, D, m, rs, ls in members:
                        sL, s1, sLM, sE, sLG, s2 = scratch[gi]
                        # t = L - m  (fp16 2x)
                        nc.vector.tensor_sub(s1[:], sL[:], m[:])
                    for gi, D, m, rs, ls in members:
                        sL, s1, sLM, sE, sLG, s2 = scratch[gi]
                        # z' = t / sigma  (fp16 2x)
                        nc.vector.tensor_mul(s1[:], s1[:], rs[:])
                    for gi, D, m, rs, ls in members:
                        sL, s1, sLM, sE, sLG, s2 = scratch[gi]
                        nc.scalar.activation(sE[:], s1[:], AF.Erf, scale=INV_SQRT2)
                    for gi, D, m, rs, ls in members:
                        sL, s1, sLM, sE, sLG, s2 = scratch[gi]
                        # v = 0.5 z'^2 + ln(sigma) - ln(phi0/sqrt2)
                        nc.vector._custom_dve(SQSB, out=s2[:], in0=s1[:],
                                              in1=ls[:], s0=0.5, s1=-LNPHI)
                    for gi, D, m, rs, ls in members:
                        sL, s1, sLM, sE, sLG, s2 = scratch[gi]
                        # LG = ln(1 + erf z)  (-inf at saturation: step right)
                        nc.scalar.activation(sLG[:], sE[:], AF.Ln, bias=1.0)
                    for gi, D, m, rs, ls in members:
                        sL, s1, sLM, sE, sLG, s2 = scratch[gi]
                        # a = LM - LG  (fp16 2x, in place)
                        nc.vector.tensor_sub(sLM[:], sLM[:], sLG[:])
                    for gi, D, m, rs, ls in members:
                        sL, s1, sLM, sE, sLG, s2 = scratch[gi]
                        # b = a - L  (fp16 2x, in place)
                        nc.vector.tensor_sub(sLM[:], sLM[:], sL[:])
                    for gi, D, m, rs, ls in members:
                        sL, s1, sLM, sE, sLG, s2 = scratch[gi]
                        # delta = b - v  (fp16 2x, in place)
                        nc.vector.tensor_sub(sLM[:], sLM[:], s2[:])
                    for gi, D, m, rs, ls in members:
                        sL, s1, sLM, sE, sLG, s2 = scratch[gi]
                        nc.vector._custom_dve(STEPD2, out=D[:], in0=sLM[:],
                                              in1=D[:], s0=s0c, s1=0.0, imm2=w)

                # ---- store (the last step already wrote the midpoint) ----
                for gi, D, m, rs, ls in members:
                    nc.sync.dma_start(out_v[gi], D[:])

    nc.finalize()
    return nc


_CACHED = {}


def _get_nc(n_per_core, fd=FD, group_size=GROUP):
    key = (n_per_core, fd, group_size)
    if key not in _CACHED:
        _CACHED[key] = _build_nc(n_per_core, fd, group_size)
    return _CACHED[key]


def kernel(params: np.ndarray, bid_prices: np.ndarray) -> np.ndarray:
    from concourse.bass_utils import run_bass_kernel_spmd

    params = np.ascontiguousarray(params, dtype=np.float32)
    bid_prices = np.ascontiguousarray(bid_prices, dtype=np.float32)
    n = bid_prices.shape[0]
    n_per_core = n // N_CORES

    nc = _get_nc(n_per_core)

    in_maps = []
    for i in range(N_CORES):
        sl = slice(i * n_per_core, (i + 1) * n_per_core)
        in_maps.append({"params": params[sl], "bids": bid_prices[sl]})

    res = run_bass_kernel_spmd(nc, in_maps, core_ids=list(range(N_CORES)))
    return np.concatenate([r["out"] for r in res.results], axis=0)


if __name__ == "__main__":
    rng = np.random.RandomState(0)
    n = N_TOTAL
    params = np.stack(
        [rng.randn(n).astype(np.float32),
         rng.uniform(0.2, 1.5, n).astype(np.float32)], axis=-1
    )
    bids = rng.uniform(0.1, 10.0, n).astype(np.float32)
    out = kernel(params=params, bid_prices=bids)
    print("out", out.shape, out.dtype, out[:8])


# revision 12
# speedup vs baseline: 2.8793x; 1.0638x over previous
"""Trainium2 Bass kernel: per-element maximization of the lognormal-CDF
surplus  s(d) = bid*(1-d)*Phi((ln(d*bid)-mu)/sigma),  d in [0,1].

Algorithm: the reference runs 20 golden-section iterations on s(d) (two
surplus evaluations per iteration).  s is log-concave in d (product of
log-concave factors composed with concave increasing maps), so s' crosses
zero exactly once and the argmax is instead found by BISECTION ON THE
SIGN OF s'(d) - one evaluation per iteration, 0.5x interval shrink
(vs 0.618x for GSS), K=7 iterations:

  s'(d) >= 0  <=>  (1-d) * B * phi0 * e^{-z^2} >= d * (1 + erf z)
     z = (ln d - m) * B,  m = mu - ln bid,  B = 1/(sigma*sqrt2),
     phi0 = 2/sqrt(pi)

evaluated in LOG space (both sides > 0):

  ln(1-d) - ln(1+erf z) - ln d - 0.5 z'^2 - ln(sigma) + ln(phi0/sqrt2) >= 0
     with z' = (ln d - m)/sigma = sqrt2 * z

which needs only {Ln, Erf} activation tables (2 table swaps/iteration)
and no Exp.  Saturation is exact: erf -> -1 gives LG = Ln(0) = -inf so
delta = +inf -> step right, matching the reference's tie behavior.

Validated against the reference output in simulation (rel-L2 5.8e-3;
the plateau is reference fp32 (1+erf) quantization noise on deep-tail
elements, not bisection resolution); measured on HW at ~3e-3 - well
under the 2e-2 gate.  Robust to 1e-3 activation-table error.

Implementation (per [128, FD] chunk):
  - Only the interval MIDPOINT D is tracked:  D' = D + (delta>=c)*w - w/2,
    one fused custom-DVE op; the final step emits the answer directly.
  - fp16 intermediates: the five chained subtractions/multiplies run as
    2-byte packed TENSOR_TENSOR ops in the DVE 2x_1p mode (2 elem/cycle);
    E stays fp32 so (1+erf) keeps fp32 resolution near saturation; D
    stays fp32 (needs 2^-K resolution near 1).
  - Custom fused DVE ops (registered at import): GSS_SQSB computes
    0.5*z'^2 + ln(sigma) - ln(phi0/sqrt2) in one pass; GSS_STEPD2 does
    compare+step+recenter in one pass; GSS_Z0/GSS_STEP0LE specialize
    iteration 0 (D_0 = 0.5 is a compile-time constant: no Ln needed, and
    ln(D_0) = ln(1-D_0) cancels in the condition).
  - GpSimd (Pool) is NOT used for elementwise work: it shares SBUF ports
    with the DVE and concurrent pool/DVE streams slow each other ~2.8x
    (measured); everything runs on DVE + ACT.
"""
import sys

sys.path.insert(0, "/opt/trn_rl_repo")

import numpy as np

N_TOTAL = 16777216
N_CORES = 8
N_PER_CORE = N_TOTAL // N_CORES  # 2097152
P = 128
FD = 2048
GROUP = 3
K_ITERS = 7

LN_HALF = float(np.log(0.5))
INV_SQRT2 = float(1.0 / np.sqrt(2.0))
# ln(phi0/sqrt2), phi0 = 2/sqrt(pi)
LNPHI = float(np.log(2.0 / np.sqrt(np.pi)) - 0.5 * np.log(2.0))

_ops_registered = {}


def _register_ops():
    """Register the fused custom-DVE ops (documented extension point:
    dve_ops.OPS + _SUB_OPCODE_FOR_NAME + CUSTOM_DVE_SPECS). uops_sha is
    computed here the same way DveOp.compile derives it."""
    if _ops_registered:
        return _ops_registered
    import concourse.dve_ops as dve_ops
    from concourse.dve_ops import DveOp, OPS
    from concourse.dve_spec import Spec, Src0, Src1, C0, C1, C2, sq, lower
    from concourse.dve_spec import _has_src1 as has_src1
    from concourse.dve_uop import DveOpSpec

    def ref_sqsb(in0, in1, s0, s1, imm2):
        return (np.float32(s0) * in0.astype(np.float32) ** 2 + in1
                + np.float32(s1)).astype(np.float32)

    def ref_stepd(in0, in1, s0, s1, imm2):
        return (in1 + (in0 >= np.float32(s1)).astype(np.float32)
                * np.float32(imm2) + np.float32(s0)).astype(np.float32)

    def ref_z0(in0, in1, s0, s1, imm2):
        return ((np.float32(s0) - in0.astype(np.float32)) * in1).astype(np.float32)

    def ref_step0le(in0, in1, s0, s1, imm2):
        return ((np.float32(s1) >= in0).astype(np.float32) * np.float32(imm2)
                + np.float32(s0)).astype(np.float32)

    defs = [
        # v = 0.5*z'^2 + ln(sigma) + (-ln(phi0/sqrt2))
        ("GSS_SQSB", sq(Src0) * C0 + Src1 + C1, ref_sqsb),
        # D' = D + (delta >= thresh)*w - w/2
        ("GSS_STEPD2", Src1 + (Src0 >= C1) * C2 + C0, ref_stepd),
        # z0 = (ln(0.5) - m) * (1/sigma)
        ("GSS_Z0", (C0 - Src0) * Src1, ref_z0),
        # D1 = (thresh >= q)*0.5 + 0.25
        ("GSS_STEP0LE", (C1 >= Src0) * C2 + C0, ref_step0le),
    ]
    for name, body, ref in defs:
        if name in dve_ops._SUB_OPCODE_FOR_NAME:
            _ops_registered[name] = next(o for o in OPS if o.name == name)
            continue
        row = dve_ops._CUSTOM_DVE_ROW_BASE + len(OPS)
        assert row < 0x20
        spec = Spec(body=body, reference=ref)
        shas = {}
        for ver in ("v3", "v4"):
            uops = lower(spec, ver=ver)
            shas[ver] = DveOpSpec(
                name=name, opcode=row, uops=uops, rd1_en=has_src1(spec)
            ).sha(ver)
        op = DveOp(name, spec, subdim=False, uops_sha=shas)
        OPS.append(op)
        dve_ops._SUB_OPCODE_FOR_NAME[name] = row
        dve_ops.CUSTOM_DVE_SPECS[name] = spec
        _ops_registered[name] = op
    return _ops_registered


def _build_nc(n_per_core, fd, group_size):
    import concourse.bass as bass  # noqa: F401
    import concourse.bacc as bacc
    import concourse.mybir as mybir
    import concourse.tile as tile

    ops = _register_ops()
    SQSB, STEPD2, Z0, STEP0LE = (
        ops["GSS_SQSB"], ops["GSS_STEPD2"], ops["GSS_Z0"], ops["GSS_STEP0LE"]
    )

    AF = mybir.ActivationFunctionType
    dt = mybir.dt.float32
    dth = mybir.dt.float16

    n_chunks = n_per_core // (P * fd)
    assert n_chunks * P * fd == n_per_core

    nc = bacc.Bacc(None, target_bir_lowering=False)

    def register_const(value: float):
        if (dt, value) in nc.const_aps.aps:
            return
        t = nc.alloc_sbuf_tensor(f"const-f32-c{len(nc.const_aps.aps)}", [128, 1], dt)
        nc.gpsimd.memset(t.ap(), value)
        nc.const_aps.aps[(dt, value)] = t.ap()

    for v in (0.0, 1.0):
        register_const(float(v))
    nc.all_engine_barrier()

    params = nc.declare_dram_parameter("params", [n_per_core, 2], dt, isOutput=False)
    bids = nc.declare_dram_parameter("bids", [n_per_core], dt, isOutput=False)
    out = nc.declare_dram_parameter("out", [n_per_core], dt, isOutput=True)

    params_v = params.rearrange("(g p f) c -> g p (f c)", p=P, f=fd)
    bids_v = bids.rearrange("(g p f) -> g p f", p=P, f=fd)
    out_v = out.rearrange("(g p f) -> g p f", p=P, f=fd)

    with tile.TileContext(nc) as tc:
        with (
            tc.tile_pool(name="st_d", bufs=group_size + 1) as p_d,
            tc.tile_pool(name="st_m", bufs=group_size + 1) as p_m,
            tc.tile_pool(name="st_rs", bufs=group_size + 1) as p_rs,
            tc.tile_pool(name="st_ls", bufs=group_size + 1) as p_ls,
            tc.tile_pool(name="sL", bufs=group_size) as p_sL,
            tc.tile_pool(name="s1", bufs=group_size) as p_s1,
            tc.tile_pool(name="sLM", bufs=group_size) as p_sLM,
            tc.tile_pool(name="sE", bufs=group_size) as p_sE,
            tc.tile_pool(name="sLG", bufs=group_size) as p_sLG,
            tc.tile_pool(name="s2", bufs=group_size) as p_s2,
            tc.tile_pool(name="pload", bufs=1) as p_pl,
        ):
            for g0 in range(0, n_chunks, group_size):
                members = []
                # ---- per-chunk setup ----
                for gi in range(g0, min(g0 + group_size, n_chunks)):
                    D = p_d.tile([P, fd], dt, tag="D")
                    m = p_m.tile([P, fd], dth, tag="m")
                    rs = p_rs.tile([P, fd], dth, tag="rs")
                    ls = p_ls.tile([P, fd], dth, tag="ls")
                    # bid -> fp32 staging, then ln in place
                    pl0 = p_pl.tile([P, fd], dt, tag="plbid", bufs=1)
                    nc.sync.dma_start(pl0[:], bids_v[gi])
                    nc.scalar.activation(pl0[:], pl0[:], AF.Ln)
                    for h in range(2):
                        pl = p_pl.tile([P, fd], dt, tag="pl", bufs=1)
                        nc.sync.dma_start(pl[:], params_v[gi, :, h * fd:(h + 1) * fd])
                        plv = pl.rearrange("p (f c) -> p f c", c=2)
                        half = slice(h * (fd // 2), (h + 1) * (fd // 2))
                        # ls = ln(sigma); rs = 1/sigma; m = mu - ln(bid)
                        nc.scalar.activation(ls[:, half], plv[:, :, 1], AF.Ln)
                        rf = p_pl.tile([P, fd // 2], dt, tag="rf", bufs=1)
                        nc.vector.reciprocal_approx_fast(out=rf[:], in_=plv[:, :, 1])
                        nc.scalar.activation(rs[:, half], rf[:], AF.Copy,
                                             scale=1.0, bias=0.0)
                        nc.vector.tensor_sub(m[:, half], plv[:, :, 0], pl0[:, half])
                    members.append((gi, D, m, rs, ls))

                # ---- iteration 0: D_0 = 0.5 (compile-time constant) ----
                # ln(1-D0) and ln(D0) cancel: condition is  LG + v0 <= 0.
                scratch = {}
                for gi, D, m, rs, ls in members:
                    s1 = p_s1.tile([P, fd], dth, tag="s1")
                    sE = p_sE.tile([P, fd], dt, tag="sE")
                    sLG = p_sLG.tile([P, fd], dth, tag="sLG")
                    s2 = p_s2.tile([P, fd], dth, tag="s2")
                    scratch[gi] = (s1, sE, sLG, s2)
                    nc.vector._custom_dve(Z0, out=s1[:], in0=m[:], in1=rs[:],
                                          s0=LN_HALF)
                for gi, D, m, rs, ls in members:
                    s1, sE, sLG, s2 = scratch[gi]
                    nc.scalar.activation(sE[:], s1[:], AF.Erf, scale=INV_SQRT2)
                for gi, D, m, rs, ls in members:
                    s1, sE, sLG, s2 = scratch[gi]
                    nc.vector._custom_dve(SQSB, out=s2[:], in0=s1[:], in1=ls[:],
                                          s0=0.5, s1=-LNPHI)
                for gi, D, m, rs, ls in members:
                    s1, sE, sLG, s2 = scratch[gi]
                    nc.scalar.activation(sLG[:], sE[:], AF.Ln, bias=1.0)
                for gi, D, m, rs, ls in members:
                    s1, sE, sLG, s2 = scratch[gi]
                    # q = LG + v0  (fp16 2x)
                    nc.vector.tensor_add(s2[:], sLG[:], s2[:])
                for gi, D, m, rs, ls in members:
                    s1, sE, sLG, s2 = scratch[gi]
                    nc.vector._custom_dve(STEP0LE, out=D[:], in0=s2[:],
                                          s0=0.25, s1=0.0, imm2=0.5)

                # ---- iterations 1..K-1 (log-space derivative sign) ----
                for k in range(1, K_ITERS):
                    w = float(2.0 ** -(k + 1))
                    s0c = float(-(2.0 ** -(k + 2)))
                    for gi, D, m, rs, ls in members:
                        sL = p_sL.tile([P, fd], dth, tag="sL")
                        s1 = p_s1.tile([P, fd], dth, tag="s1")
                        sLM = p_sLM.tile([P, fd], dth, tag="sLM")
                        sE = p_sE.tile([P, fd], dt, tag="sE")
                        sLG = p_sLG.tile([P, fd], dth, tag="sLG")
                        s2 = p_s2.tile([P, fd], dth, tag="s2")
                        scratch[gi] = (sL, s1, sLM, sE, sLG, s2)
                        nc.scalar.activation(sL[:], D[:], AF.Ln)
                    for gi, D, m, rs, ls in members:
                        sL, s1, sLM, sE, sLG, s2 = scratch[gi]
                        # LM = ln(1 - D)
                        nc.scalar.activation(sLM[:], D[:], AF.Ln, scale=-1.0,
                                             bias=1.0)
                    for gi, D, m, rs, ls in members:
                        sL, s1, sLM, sE, sLG, s2 = scratch[gi]
                        # t = L - m  (fp16 2x)
                        nc.vector.tensor_sub(s1[:], sL[:], m[:])
                    for gi, D, m, rs, ls in members:
                        sL, s1, sLM, sE, sLG, s2 = scratch[gi]
                        # z' = t / sigma  (fp16 2x)
                        nc.vector.tensor_mul(s1[:], s1[:], rs[:])
                    for gi, D, m, rs, ls in members:
                        sL, s1, sLM, sE, sLG, s2 = scratch[gi]
                        nc.scalar.activation(sE[:], s1[:], AF.Erf, scale=INV_SQRT2)
                    for gi, D, m, rs, ls in members:
                        sL, s1, sLM, sE, sLG, s2 = scratch[gi]
                        # v = 0.5 z'^2 + ln(sigma) - ln(phi0/sqrt2)
                        nc.vector._custom_dve(SQSB, out=s2[:], in0=s1[:],
                                              in1=ls[:], s0=0.5, s1=-LNPHI)
                    for gi, D, m, rs, ls in members:
                        sL, s1, sLM, sE, sLG, s2 = scratch[gi]
                        # LG = ln(1 + erf z)  (-inf at saturation: step right)
                        nc.scalar.activation(sLG[:], sE[:], AF.Ln, bias=1.0)
                    for gi, D, m, rs, ls in members:
                        sL, s1, sLM, sE, sLG, s2 = scratch[gi]
                        # a = LM - LG  (fp16 2x, in place)
                        nc.vector.tensor_sub(sLM[:], sLM[:], sLG[:])
                    for gi, D, m, rs, ls in members:
                        sL, s1, sLM, sE, sLG, s2 = scratch[gi]
                        # b = a - L  (fp16 2x, in place)
                        nc.vector.tensor_sub(sLM[:], sLM[:], sL[:])
                    for gi, D, m, rs, ls in members:
                        sL, s1, sLM, sE, sLG, s2 = scratch[gi]
                        # delta = b - v  (fp16 2x, in place)
                        nc.vector.tensor_sub(sLM[:], sLM[:], s2[:])
                    for gi, D, m, rs, ls in members:
                        sL, s1, sLM, sE, sLG, s2 = scratch[gi]
                        nc.vector._custom_dve(STEPD2, out=D[:], in0=sLM[:],
                                              in1=D[:], s0=s0c, s1=0.0, imm2=w)

                # ---- store (the last step already wrote the midpoint) ----
                for gi, D, m, rs, ls in members:
                    nc.sync.dma_start(out_v[gi], D[:])

    nc.finalize()
    return nc


_CACHED = {}


def _get_nc(n_per_core, fd=FD, group_size=GROUP):
    key = (n_per_core, fd, group_size)
    if key not in _CACHED:
        _CACHED[key] = _build_nc(n_per_core, fd, group_size)
    return _CACHED[key]


def kernel(params: np.ndarray, bid_prices: np.ndarray) -> np.ndarray:
    from concourse.bass_utils import run_bass_kernel_spmd

    params = np.ascontiguousarray(params, dtype=np.float32)
    bid_prices = np.ascontiguousarray(bid_prices, dtype=np.float32)
    n = bid_prices.shape[0]
    n_per_core = n // N_CORES

    nc = _get_nc(n_per_core)

    in_maps = []
    for i in range(N_CORES):
        sl = slice(i * n_per_core, (i + 1) * n_per_core)
        in_maps.append({"params": params[sl], "bids": bid_prices[sl]})

    res = run_bass_kernel_spmd(nc, in_maps, core_ids=list(range(N_CORES)))
    return np.concatenate([r["out"] for r in res.results], axis=0)


if __name__ == "__main__":
    rng = np.random.RandomState(0)
    n = N_TOTAL
    params = np.stack(
        [rng.randn(n).astype(np.float32),
         rng.uniform(0.2, 1.5, n).astype(np.float32)], axis=-1
    )
    bids = rng.uniform(0.1, 10.0, n).astype(np.float32)
    out = kernel(params=params, bid_prices=bids)
    print("out", out.shape, out.dtype, out[:8])


# revision 14
# speedup vs baseline: 3.4501x; 1.1982x over previous
"""Trainium2 Bass kernel: per-element maximization of the lognormal-CDF
surplus  s(d) = bid*(1-d)*Phi((ln(d*bid)-mu)/sigma),  d in [0,1].

Algorithm: the reference runs 20 golden-section iterations on s(d) (two
surplus evaluations per iteration).  s is log-concave in d (product of
log-concave factors composed with concave increasing maps), so s' crosses
zero exactly once and the argmax is instead found by BISECTION ON THE
SIGN OF s'(d) - one evaluation per iteration, 0.5x interval shrink
(vs 0.618x for GSS), K=7 iterations:

  s'(d) >= 0  <=>  (1-d) * B * phi0 * e^{-z^2} >= d * (1 + erf z)
     z = (ln d - m) * B,  m = mu - ln bid,  B = 1/(sigma*sqrt2),
     phi0 = 2/sqrt(pi)

evaluated in LOG space (both sides > 0):

  ln(1-d) - ln(1+erf z) - ln d - 0.5 z'^2 - ln(sigma) + ln(phi0/sqrt2) >= 0
     with z' = (ln d - m)/sigma = sqrt2 * z

which needs only {Ln, Erf} activation tables (2 table swaps/iteration)
and no Exp.  Saturation is exact: erf -> -1 gives LG = Ln(0) = -inf so
delta = +inf -> step right, matching the reference's tie behavior.

Validated against the reference output in simulation (rel-L2 5.8e-3;
the plateau is reference fp32 (1+erf) quantization noise on deep-tail
elements, not bisection resolution); measured on HW at ~3e-3 - well
under the 2e-2 gate.  Robust to 1e-3 activation-table error.

Implementation (per [128, FD] chunk):
  - Only the interval MIDPOINT D is tracked:  D' = D + (delta>=c)*w - w/2,
    one fused custom-DVE op; the final step emits the answer directly.
  - fp16 intermediates: the five chained subtractions/multiplies run as
    2-byte packed TENSOR_TENSOR ops in the DVE 2x_1p mode (2 elem/cycle);
    E stays fp32 so (1+erf) keeps fp32 resolution near saturation; D
    stays fp32 (needs 2^-K resolution near 1).
  - Custom fused DVE ops (registered at import): GSS_SQSB computes
    0.5*z'^2 + ln(sigma) - ln(phi0/sqrt2) in one pass; GSS_STEPD2 does
    compare+step+recenter in one pass; GSS_Z0/GSS_STEP0LE specialize
    iteration 0 (D_0 = 0.5 is a compile-time constant: no Ln needed, and
    ln(D_0) = ln(1-D_0) cancels in the condition).
  - GpSimd (Pool) is NOT used for elementwise work: it shares SBUF ports
    with the DVE and concurrent pool/DVE streams slow each other ~2.8x
    (measured); everything runs on DVE + ACT.
"""
import sys

sys.path.insert(0, "/opt/trn_rl_repo")

import numpy as np

N_TOTAL = 16777216
N_CORES = 8
N_PER_CORE = N_TOTAL // N_CORES  # 2097152
P = 128
FD = 2048
GROUP = 3
K_ITERS = 5

LN_HALF = float(np.log(0.5))
INV_SQRT2 = float(1.0 / np.sqrt(2.0))
# ln(phi0/sqrt2), phi0 = 2/sqrt(pi)
LNPHI = float(np.log(2.0 / np.sqrt(np.pi)) - 0.5 * np.log(2.0))

_ops_registered = {}


def _register_ops():
    """Register the fused custom-DVE ops (documented extension point:
    dve_ops.OPS + _SUB_OPCODE_FOR_NAME + CUSTOM_DVE_SPECS). uops_sha is
    computed here the same way DveOp.compile derives it."""
    if _ops_registered:
        return _ops_registered
    import concourse.dve_ops as dve_ops
    from concourse.dve_ops import DveOp, OPS
    from concourse.dve_spec import (Spec, Src0, Src1, C0, C1, C2, Zero,
                                    One, sq, minn, maxx, lower)
    from concourse.dve_spec import _has_src1 as has_src1
    from concourse.dve_uop import DveOpSpec

    def ref_sqsb(in0, in1, s0, s1, imm2):
        return (np.float32(s0) * in0.astype(np.float32) ** 2 + in1
                + np.float32(s1)).astype(np.float32)

    def ref_stepd(in0, in1, s0, s1, imm2):
        return (in1 + (in0 >= np.float32(s1)).astype(np.float32)
                * np.float32(imm2) + np.float32(s0)).astype(np.float32)

    def ref_z0(in0, in1, s0, s1, imm2):
        return ((np.float32(s0) - in0.astype(np.float32)) * in1).astype(np.float32)

    def ref_step0le(in0, in1, s0, s1, imm2):
        return ((np.float32(s1) >= in0).astype(np.float32) * np.float32(imm2)
                + np.float32(s0)).astype(np.float32)

    def ref_step0ge(in0, in1, s0, s1, imm2):
        return ((in0 >= np.float32(s1)).astype(np.float32) * np.float32(imm2)
                + np.float32(s0)).astype(np.float32)

    def ref_qclamp(in0, in1, s0, s1, imm2):
        q = in0.astype(np.float32) * in1.astype(np.float32)
        q = np.where(np.isnan(q), -1.0, q)
        return np.clip(q, -1.0, 1.0).astype(np.float32)

    defs = [
        # v = 0.5*z'^2 + ln(sigma) + (-ln(phi0/sqrt2))
        ("GSS_SQSB", sq(Src0) * C0 + Src1 + C1, ref_sqsb),
        # D' = D + (delta >= thresh)*w - w/2
        ("GSS_STEPD2", Src1 + (Src0 >= C1) * C2 + C0, ref_stepd),
        # z0 = (ln(0.5) - m) * (1/sigma)
        ("GSS_Z0", (C0 - Src0) * Src1, ref_z0),
        # D1 = (thresh >= q)*0.5 + 0.25
        ("GSS_STEP0LE", (C1 >= Src0) * C2 + C0, ref_step0le),
        # dDp = (deltam >= thresh)*2^-(K-1) - 2^-K   (signed last step)
        ("GSS_STEP0GE", (Src0 >= C1) * C2 + C0, ref_step0ge),
        # qc = clamp(delta * r, -1, 1)  (DVE max/min absorb NaN)
        ("GSS_QCLAMP", minn(maxx(Src0 * Src1, Zero - One), One), ref_qclamp),
    ]
    for name, body, ref in defs:
        if name in dve_ops._SUB_OPCODE_FOR_NAME:
            _ops_registered[name] = next(o for o in OPS if o.name == name)
            continue
        row = dve_ops._CUSTOM_DVE_ROW_BASE + len(OPS)
        assert row < 0x20
        spec = Spec(body=body, reference=ref)
        shas = {}
        for ver in ("v3", "v4"):
            uops = lower(spec, ver=ver)
            shas[ver] = DveOpSpec(
                name=name, opcode=row, uops=uops, rd1_en=has_src1(spec)
            ).sha(ver)
        op = DveOp(name, spec, subdim=False, uops_sha=shas)
        OPS.append(op)
        dve_ops._SUB_OPCODE_FOR_NAME[name] = row
        dve_ops.CUSTOM_DVE_SPECS[name] = spec
        _ops_registered[name] = op
    return _ops_registered


def _build_nc(n_per_core, fd, group_size):
    import concourse.bass as bass  # noqa: F401
    import concourse.bacc as bacc
    import concourse.mybir as mybir
    import concourse.tile as tile

    ops = _register_ops()
    SQSB, STEPD2, Z0, STEP0LE, STEP0GE, QCLAMP = (
        ops["GSS_SQSB"], ops["GSS_STEPD2"], ops["GSS_Z0"], ops["GSS_STEP0LE"],
        ops["GSS_STEP0GE"], ops["GSS_QCLAMP"]
    )

    AF = mybir.ActivationFunctionType
    dt = mybir.dt.float32
    dth = mybir.dt.float16

    n_chunks = n_per_core // (P * fd)
    assert n_chunks * P * fd == n_per_core

    nc = bacc.Bacc(None, target_bir_lowering=False)

    def register_const(value: float):
        if (dt, value) in nc.const_aps.aps:
            return
        t = nc.alloc_sbuf_tensor(f"const-f32-c{len(nc.const_aps.aps)}", [128, 1], dt)
        nc.gpsimd.memset(t.ap(), value)
        nc.const_aps.aps[(dt, value)] = t.ap()

    for v in (0.0, 1.0):
        register_const(float(v))
    nc.all_engine_barrier()

    params = nc.declare_dram_parameter("params", [n_per_core, 2], dt, isOutput=False)
    bids = nc.declare_dram_parameter("bids", [n_per_core], dt, isOutput=False)
    out = nc.declare_dram_parameter("out", [n_per_core], dt, isOutput=True)

    params_v = params.rearrange("(g p f) c -> g p (f c)", p=P, f=fd)
    bids_v = bids.rearrange("(g p f) -> g p f", p=P, f=fd)
    out_v = out.rearrange("(g p f) -> g p f", p=P, f=fd)

    with tile.TileContext(nc) as tc:
        with (
            tc.tile_pool(name="st_d", bufs=group_size + 1) as p_d,
            tc.tile_pool(name="st_m", bufs=group_size + 1) as p_m,
            tc.tile_pool(name="st_rs", bufs=group_size + 1) as p_rs,
            tc.tile_pool(name="st_ls", bufs=group_size + 1) as p_ls,
            tc.tile_pool(name="sL", bufs=group_size) as p_sL,
            tc.tile_pool(name="s1", bufs=group_size) as p_s1,
            tc.tile_pool(name="sLM", bufs=2 * group_size) as p_sLM,
            tc.tile_pool(name="sE", bufs=group_size) as p_sE,
            tc.tile_pool(name="sLG", bufs=group_size) as p_sLG,
            tc.tile_pool(name="pload", bufs=1) as p_pl,
        ):
            for g0 in range(0, n_chunks, group_size):
                members = []
                # ---- per-chunk setup ----
                for gi in range(g0, min(g0 + group_size, n_chunks)):
                    D = p_d.tile([P, fd], dt, tag="D")
                    m = p_m.tile([P, fd], dth, tag="m")
                    rs = p_rs.tile([P, fd], dth, tag="rs")
                    ls = p_ls.tile([P, fd], dth, tag="ls")
                    # bid -> fp32 staging, then ln in place
                    pl0 = p_pl.tile([P, fd], dt, tag="plbid", bufs=1)
                    nc.sync.dma_start(pl0[:], bids_v[gi])
                    nc.scalar.activation(pl0[:], pl0[:], AF.Ln)
                    for h in range(2):
                        pl = p_pl.tile([P, fd], dt, tag="pl", bufs=1)
                        nc.sync.dma_start(pl[:], params_v[gi, :, h * fd:(h + 1) * fd])
                        plv = pl.rearrange("p (f c) -> p f c", c=2)
                        half = slice(h * (fd // 2), (h + 1) * (fd // 2))
                        # ls = ln(sigma); rs = 1/sigma; m = mu - ln(bid)
                        nc.scalar.activation(ls[:, half], plv[:, :, 1], AF.Ln)
                        rf = p_pl.tile([P, fd // 2], dt, tag="rf", bufs=1)
                        nc.vector.reciprocal_approx_fast(out=rf[:], in_=plv[:, :, 1])
                        nc.scalar.activation(rs[:, half], rf[:], AF.Copy,
                                             scale=1.0, bias=0.0)
                        nc.vector.tensor_sub(m[:, half], plv[:, :, 0], pl0[:, half])
                    members.append((gi, D, m, rs, ls))

                # ---- iteration 0: D_0 = 0.5 (compile-time constant) ----
                # ln(1-D0) and ln(D0) cancel: condition is  LG + v0 <= 0.
                scratch = {}
                for gi, D, m, rs, ls in members:
                    s1 = p_s1.tile([P, fd], dth, tag="s1")
                    sE = p_sE.tile([P, fd], dt, tag="sE")
                    sLG = p_sLG.tile([P, fd], dth, tag="sLG")
                    scratch[gi] = (s1, sE, sLG)
                    nc.vector._custom_dve(Z0, out=s1[:], in0=m[:], in1=rs[:],
                                          s0=LN_HALF)
                for gi, D, m, rs, ls in members:
                    s1, sE, sLG = scratch[gi]
                    nc.scalar.activation(sE[:], s1[:], AF.Erf, scale=INV_SQRT2)
                for gi, D, m, rs, ls in members:
                    s1, sE, sLG = scratch[gi]
                    # v0 = 0.5 z0^2 + ls - LNPHI (in place; Erf already read s1)
                    nc.vector._custom_dve(SQSB, out=s1[:], in0=s1[:], in1=ls[:],
                                          s0=0.5, s1=-LNPHI)
                for gi, D, m, rs, ls in members:
                    s1, sE, sLG = scratch[gi]
                    nc.scalar.activation(sLG[:], sE[:], AF.Ln, bias=1.0)
                for gi, D, m, rs, ls in members:
                    s1, sE, sLG = scratch[gi]
                    # q = LG + v0  (fp16 2x)
                    nc.vector.tensor_add(s1[:], sLG[:], s1[:])
                for gi, D, m, rs, ls in members:
                    s1, sE, sLG = scratch[gi]
                    nc.vector._custom_dve(STEP0LE, out=D[:], in0=s1[:],
                                          s0=0.25, s1=0.0, imm2=0.5)

                # ---- iterations 1..K-1 (log-space derivative sign) ----
                # The last iteration only evaluates delta; a clamped secant
                # step over the last two (D, delta) pairs replaces ~2 more
                # bisection iterations.
                prev_delta = {}
                for k in range(1, K_ITERS):
                    w = float(2.0 ** -(k + 1))
                    s0c = float(-(2.0 ** -(k + 2)))
                    last = k == K_ITERS - 1
                    for gi, D, m, rs, ls in members:
                        sL = p_sL.tile([P, fd], dth, tag="sL")
                        s1 = p_s1.tile([P, fd], dth, tag="s1")
                        sLM = p_sLM.tile([P, fd], dth, tag="sLM")
                        sE = p_sE.tile([P, fd], dt, tag="sE")
                        sLG = p_sLG.tile([P, fd], dth, tag="sLG")
                        scratch[gi] = (sL, s1, sLM, sE, sLG)
                        nc.scalar.activation(sL[:], D[:], AF.Ln)
                    for gi, D, m, rs, ls in members:
                        sL, s1, sLM, sE, sLG = scratch[gi]
                        # LM = ln(1 - D)
                        nc.scalar.activation(sLM[:], D[:], AF.Ln, scale=-1.0,
                                             bias=1.0)
                    for gi, D, m, rs, ls in members:
                        sL, s1, sLM, sE, sLG = scratch[gi]
                        # t = L - m  (fp16 2x)
                        nc.vector.tensor_sub(s1[:], sL[:], m[:])
                    for gi, D, m, rs, ls in members:
                        sL, s1, sLM, sE, sLG = scratch[gi]
                        # z' = t / sigma  (fp16 2x)
                        nc.vector.tensor_mul(s1[:], s1[:], rs[:])
                    for gi, D, m, rs, ls in members:
                        sL, s1, sLM, sE, sLG = scratch[gi]
                        nc.scalar.activation(sE[:], s1[:], AF.Erf, scale=INV_SQRT2)
                    for gi, D, m, rs, ls in members:
                        sL, s1, sLM, sE, sLG = scratch[gi]
                        # v = 0.5 z'^2 + ln(sigma) - ln(phi0/sqrt2)
                        # (in place; Erf already read s1)
                        nc.vector._custom_dve(SQSB, out=s1[:], in0=s1[:],
                                              in1=ls[:], s0=0.5, s1=-LNPHI)
                    for gi, D, m, rs, ls in members:
                        sL, s1, sLM, sE, sLG = scratch[gi]
                        # LG = ln(1 + erf z)  (-inf at saturation: step right)
                        nc.scalar.activation(sLG[:], sE[:], AF.Ln, bias=1.0)
                    for gi, D, m, rs, ls in members:
                        sL, s1, sLM, sE, sLG = scratch[gi]
                        # a = LM - LG  (fp16 2x, in place)
                        nc.vector.tensor_sub(sLM[:], sLM[:], sLG[:])
                    for gi, D, m, rs, ls in members:
                        sL, s1, sLM, sE, sLG = scratch[gi]
                        # b = a - L  (fp16 2x, in place)
                        nc.vector.tensor_sub(sLM[:], sLM[:], sL[:])
                    for gi, D, m, rs, ls in members:
                        sL, s1, sLM, sE, sLG = scratch[gi]
                        # delta = b - v  (fp16 2x, in place)
                        nc.vector.tensor_sub(sLM[:], sLM[:], s1[:])
                    if not last:
                        for gi, D, m, rs, ls in members:
                            sL, s1, sLM, sE, sLG = scratch[gi]
                            nc.vector._custom_dve(STEPD2, out=D[:], in0=sLM[:],
                                                  in1=D[:], s0=s0c, s1=0.0,
                                                  imm2=w)
                        prev_delta = {gi: scratch[gi][2] for gi in scratch}

                # ---- secant tail: D* = D - clamp(delta/dd, -1, 1) * dDp ----
                wK1 = float(2.0 ** -(K_ITERS - 1))   # |step| at k=K-2 was wK1/2
                for gi, D, m, rs, ls in members:
                    sL, s1, sLM, sE, sLG = scratch[gi]
                    # dDp = (deltam >= 0)*2^-(K-1) - 2^-K  (= +-2^-K, signed)
                    nc.vector._custom_dve(STEP0GE, out=sLG[:],
                                          in0=prev_delta[gi][:],
                                          s0=-wK1 / 2.0, s1=0.0, imm2=wK1)
                for gi, D, m, rs, ls in members:
                    sL, s1, sLM, sE, sLG = scratch[gi]
                    # dd = delta - deltam  (fp32)
                    nc.vector.tensor_sub(sE[:], sLM[:], prev_delta[gi][:])
                for gi, D, m, rs, ls in members:
                    sL, s1, sLM, sE, sLG = scratch[gi]
                    nc.vector.reciprocal_approx_fast(out=sE[:], in_=sE[:])
                for gi, D, m, rs, ls in members:
                    sL, s1, sLM, sE, sLG = scratch[gi]
                    # qc = clamp(delta / dd, -1, 1); NaN/garbage bounded
                    nc.vector._custom_dve(QCLAMP, out=s1[:], in0=sLM[:],
                                          in1=sE[:])
                for gi, D, m, rs, ls in members:
                    sL, s1, sLM, sE, sLG = scratch[gi]
                    # c = qc * dDp  (fp16 2x)
                    nc.vector.tensor_mul(s1[:], s1[:], sLG[:])
                for gi, D, m, rs, ls in members:
                    sL, s1, sLM, sE, sLG = scratch[gi]
                    nc.vector.tensor_sub(D[:], D[:], s1[:])

                # ---- store ----
                for gi, D, m, rs, ls in members:
                    nc.sync.dma_start(out_v[gi], D[:])

    nc.finalize()
    return nc


_CACHED = {}


def _get_nc(n_per_core, fd=FD, group_size=GROUP):
    key = (n_per_core, fd, group_size)
    if key not in _CACHED:
        _CACHED[key] = _build_nc(n_per_core, fd, group_size)
    return _CACHED[key]


def kernel(params: np.ndarray, bid_prices: np.ndarray) -> np.ndarray:
    from concourse.bass_utils import run_bass_kernel_spmd

    params = np.ascontiguousarray(params, dtype=np.float32)
    bid_prices = np.ascontiguousarray(bid_prices, dtype=np.float32)
    n = bid_prices.shape[0]
    n_per_core = n // N_CORES

    nc = _get_nc(n_per_core)

    in_maps = []
    for i in range(N_CORES):
        sl = slice(i * n_per_core, (i + 1) * n_per_core)
        in_maps.append({"params": params[sl], "bids": bid_prices[sl]})

    res = run_bass_kernel_spmd(nc, in_maps, core_ids=list(range(N_CORES)))
    return np.concatenate([r["out"] for r in res.results], axis=0)


if __name__ == "__main__":
    rng = np.random.RandomState(0)
    n = N_TOTAL
    params = np.stack(
        [rng.randn(n).astype(np.float32),
         rng.uniform(0.2, 1.5, n).astype(np.float32)], axis=-1
    )
    bids = rng.uniform(0.1, 10.0, n).astype(np.float32)
    out = kernel(params=params, bid_prices=bids)
    print("out", out.shape, out.dtype, out[:8])
